# revision 14
# baseline (speedup 1.0000x reference)
"""GQA FlashAttention (RMSNorm QK + RoPE, causal) on 8 TRN2 NeuronCores.

Sharding: tensor-parallel over heads (core c owns q-heads 4c..4c+3 and
kv-head c; the GQA group is fully local). Head-chunked AllToAlls re-shard
the attention output from head-parallel to row-parallel; each core then
computes its 256 output rows against the full Wo.

v5 vs v4:
- Row-packed scores: the K=64 scores matmuls for a pair of heads run
  concurrently in PE row-groups 0 and 64 (kt duplicated to both halves,
  qt stored pair-stacked), halving attention PE occupancy for the same
  ACT cost. This also removes the cross-partition RoPE writes.
- Two-pass projections (Q pass then KV pass over SBUF-resident x tiles)
  cut the live accumulator banks from 3 to 2, freeing PSUM for a second
  PV bank: 2 acc + 4 scores + 2 pv = 8 banks.
- Out projection runs inside the same pool scope: its half-0
  accumulation (chunks 0/1 of the AllToAll) executes in the freed acc
  banks while chunks 2/3 are still in flight.
"""

import sys

sys.path.insert(0, "/opt/trn_rl_repo")

import numpy as np
import ml_dtypes
import concourse.bass as bass  # noqa: F401
import concourse.tile as tile
from concourse import mybir, bacc
import concourse.bacc as bacc_mod
from concourse.bass_utils import run_bass_kernel_spmd
from concourse.hw_specs import get_activation_tables as _orig_get_tables
from concourse.masks import make_identity

N_CORES = 8
D_IN = 2048
SEQ = 2048
N_HEADS = 32
N_KV = 8
HD = 64
HPC = N_HEADS // N_CORES  # 4 q heads per core
EPS = 1e-6
NEG = -1.0e9

F32 = mybir.dt.float32
F32R = mybir.dt.float32r
BF16 = mybir.dt.bfloat16
BF16_NP = ml_dtypes.bfloat16

KT_TILES = D_IN // 128
QB = 512
NQB = SEQ // QB  # 4
ROWS_PER_CORE = SEQ // N_CORES  # 256
AF = mybir.ActivationFunctionType

_ONE_TABLE = "natural_log_exp_and_others"


def _pinned_tables(arch):
    tabs = _orig_get_tables(arch)
    return {n: (fs if n == _ONE_TABLE else set()) for n, fs in tabs.items()}


def _build():
    bacc_mod.get_activation_tables = _pinned_tables
    nc = bacc.Bacc(num_devices=N_CORES)

    xT = nc.dram_tensor("xT", [D_IN, SEQ], BF16, kind="ExternalInput")
    wq = nc.dram_tensor("wq", [128, KT_TILES, HPC * HD], BF16, kind="ExternalInput")
    wkv = nc.dram_tensor("wkv", [128, KT_TILES, 2 * HD], BF16, kind="ExternalInput")
    wo = nc.dram_tensor("wo", [128, KT_TILES, D_IN], BF16, kind="ExternalInput")
    coswq = nc.dram_tensor("coswq", [128, SEQ], F32, kind="ExternalInput")
    sinwq = nc.dram_tensor("sinwq", [128, SEQ], F32, kind="ExternalInput")
    coswk = nc.dram_tensor("coswk", [64, SEQ], F32, kind="ExternalInput")
    sinwk = nc.dram_tensor("sinwk", [64, SEQ], F32, kind="ExternalInput")
    tri = nc.dram_tensor("tri", [128, 128], F32, kind="ExternalInput")
    sel16 = nc.dram_tensor("sel16", [2 * N_CORES, 2 * N_CORES, 128], F32R, kind="ExternalInput")
    onesblk_in = nc.dram_tensor("onesblk", [128, 128], F32R, kind="ExternalInput")

    out = nc.dram_tensor("out", [ROWS_PER_CORE, D_IN], F32, kind="ExternalOutput")

    with tile.TileContext(nc) as tc:
        with (
            tc.tile_pool(name="persist", bufs=1) as pers,
            tc.tile_pool(name="dram", bufs=1, space="DRAM") as dram,
        ):
            # ---- persistent SBUF preloads (contiguous, host-transposed) ----
            wq_sb = pers.tile([128, KT_TILES, HPC * HD], BF16)
            nc.sync.dma_start(wq_sb[:], wq[:])
            wkv_sb = pers.tile([128, KT_TILES, 2 * HD], BF16)
            nc.sync.dma_start(wkv_sb[:], wkv[:])

            cq_sb = pers.tile([128, SEQ], F32)
            sq_sb = pers.tile([128, SEQ], F32)
            ck_sb = pers.tile([64, SEQ], F32)
            sk_sb = pers.tile([64, SEQ], F32)
            nc.gpsimd.dma_start(cq_sb[:], coswq[:])
            nc.gpsimd.dma_start(sq_sb[:], sinwq[:])
            nc.gpsimd.dma_start(ck_sb[:], coswk[:])
            nc.gpsimd.dma_start(sk_sb[:], sinwk[:])
            tri_sb = pers.tile([128, 128], F32)
            nc.gpsimd.dma_start(tri_sb[:], tri[:])
            onesblk = pers.tile([128, 128], F32R)
            nc.gpsimd.dma_start(onesblk[:], onesblk_in[:])

            wo_sb = pers.tile([128, KT_TILES, D_IN], BF16)  # 8 MB
            sel_sb = pers.tile([2 * N_CORES, 2 * N_CORES, 128], F32R)

            ident = pers.tile([128, 128], F32)
            make_identity(nc, ident[:])
            eps_sb = pers.tile([128, 1], F32)
            nc.vector.memset(eps_sb[:], EPS)

            # pair-stacked q (pair p holds heads 2p/2p+1 in partition halves)
            qt = [pers.tile([128, 2, QB], BF16, name=f"qt{j}") for j in range(NQB)]
            # k duplicated into both partition halves for row-group packing
            kt = [pers.tile([128, QB], BF16, name=f"kt{j}") for j in range(NQB)]
            vaug = [pers.tile([128, 4, HD + 1], BF16, name=f"va{j}") for j in range(NQB)]

            a2a_in = [
                dram.tile([N_CORES, HD + 1, ROWS_PER_CORE], F32, name=f"a2ai{h}")
                for h in range(HPC)
            ]
            a2a_out = [
                dram.tile([N_CORES, HD + 1, ROWS_PER_CORE], F32, name=f"a2ao{h}")
                for h in range(HPC)
            ]

            # ============ fused projections + attention ====================
            with (
                tc.tile_pool(name="xt", bufs=18) as xp,
                tc.tile_pool(name="acc", bufs=2, space="PSUM") as psA,
                tc.tile_pool(name="sc", bufs=2, space="PSUM") as psB,
                tc.tile_pool(name="pv", bufs=2, space="PSUM") as psPV,
                tc.tile_pool(name="work", bufs=2) as t1,
                tc.tile_pool(name="ptp", bufs=3) as ptp,
            ):

                def norm_rope(j, raw_psum, idx):
                    """Evict + rmsnorm + rope one accumulator. idx 0/1 = q
                    pairs, idx 2 = kv. Generator (yields mid-chain)."""
                    sl = slice(QB * j, QB * j + QB)
                    is_kv = idx == 2
                    rows = slice(0, 64) if is_kv else slice(0, 128)
                    rawsb = t1.tile([128, QB], F32, tag="rawsb")
                    nc.vector.tensor_copy(rawsb[:], raw_psum[:])
                    sq = t1.tile([128, QB], F32R, tag="sq")
                    nc.vector.tensor_mul(sq[:], rawsb[:], rawsb[:])
                    psn = psB.tile([128, 2, QB], F32, tag="sc", name=f"psn{idx}_{j}")
                    nc.tensor.matmul(psn[:, 0, :], onesblk[:], sq[:], start=True, stop=True)
                    lnv = t1.tile([128, QB], F32, tag="lnv", bufs=1)
                    nc.scalar.activation(
                        out=lnv[rows, :], in_=psn[rows, 0, :],
                        func=AF.Ln, bias=eps_sb[rows, :], scale=1.0 / HD,
                    )
                    rcp = t1.tile([128, QB], F32, tag="rcp", bufs=1)
                    nc.scalar.activation(
                        out=rcp[rows, :], in_=lnv[rows, :], func=AF.Exp, scale=-0.5,
                    )
                    yield
                    tn = t1.tile([128, QB], F32, tag="tn")
                    nc.vector.tensor_mul(tn[rows, :], rawsb[rows, :], rcp[rows, :])
                    rot = t1.tile([128, QB], F32, tag="rot")
                    nh = 1 if is_kv else 2
                    for b in range(nh):
                        o = 64 * b
                        nc.vector.tensor_copy(rot[o : o + 32, :], tn[o + 32 : o + 64, :])
                        nc.vector.tensor_copy(rot[o + 32 : o + 64, :], tn[o : o + 32, :])
                    if is_kv:
                        tcs = t1.tile([64, QB], F32, tag="tcs", bufs=1)
                        nc.vector.tensor_mul(tcs[:], tn[0:64, :], ck_sb[:, sl])
                        nc.vector.tensor_mul(rot[0:64, :], rot[0:64, :], sk_sb[:, sl])
                        nc.vector.tensor_add(kt[j][0:64, :], tcs[:], rot[0:64, :])
                        nc.vector.tensor_copy(kt[j][64:128, :], kt[j][0:64, :])
                        vt = t1.tile([64, QB], F32, tag="vt", bufs=1)
                        nc.vector.tensor_copy(vt[:], rawsb[64:128, :])
                        for d in range(4):
                            psv = psB.tile([128, 2, QB], F32, tag="sc", name=f"psv{j}_{d}")
                            nc.tensor.transpose(
                                psv[:, 0, 0:64],
                                vt[:, 128 * d : 128 * d + 128],
                                ident[0:64, 0:64],
                            )
                            nc.vector.tensor_copy(vaug[j][:, d, 0:HD], psv[:, 0, 0:64])
                            nc.vector.memset(vaug[j][:, d, HD : HD + 1], 1.0)
                            if d == 1:
                                yield
                    else:
                        tc2 = t1.tile([128, QB], F32, tag="tc2")
                        nc.vector.tensor_mul(tc2[:], tn[:], cq_sb[:, sl])
                        nc.vector.tensor_mul(rot[:], rot[:], sq_sb[:, sl])
                        nc.vector.tensor_add(qt[j][:, idx, :], tc2[:], rot[:])
                    yield

                def emit_proj(j):
                    """Two-pass projection: Q (2 banks) then KV (1 bank),
                    sharing one SBUF-resident set of x tiles."""
                    sl = slice(QB * j, QB * j + QB)
                    xts = []
                    accq = [
                        psA.tile([128, QB], F32, tag="acc", name=f"accq{i}_{j}")
                        for i in range(2)
                    ]
                    for k in range(KT_TILES):
                        xt = xp.tile([128, QB], BF16, tag="xt", name=f"xt{j}_{k}")
                        xts.append(xt)
                        nc.sync.dma_start(xt[:], xT[128 * k : 128 * k + 128, sl])
                        st = k == 0
                        sp = k == KT_TILES - 1
                        nc.tensor.matmul(accq[0][:], wq_sb[:, k, 0:128], xt[:], start=st, stop=sp)
                        nc.tensor.matmul(accq[1][:], wq_sb[:, k, 128:256], xt[:], start=st, stop=sp)
                        if k % 2 == 1:
                            yield
                    yield from norm_rope(j, accq[0], 0)
                    yield from norm_rope(j, accq[1], 1)
                    acckv = psA.tile([128, QB], F32, tag="acc", name=f"acckv_{j}")
                    for k in range(KT_TILES):
                        st = k == 0
                        sp = k == KT_TILES - 1
                        nc.tensor.matmul(acckv[:], wkv_sb[:, k, :], xts[k][:], start=st, stop=sp)
                        if k % 4 == 3:
                            yield
                    yield from norm_rope(j, acckv, 2)

                def emit_att(j):
                    """Attention for block j, head-pair packed: scores for
                    heads 2p/2p+1 run concurrently in PE row groups 0/64.
                    Yields after each unit; yields (pair index) after a
                    pair's output is fully stored."""
                    for p in range(2):
                        pvs = [
                            psPV.tile([128, QB], F32, tag="pv", name=f"pv{j}_{p}_{u}")
                            for u in range(2)
                        ]
                        ntile = 4 * j + 4
                        for t in range(ntile):
                            jj, d = t // 4, t % 4
                            diag = jj == j
                            n0 = 128 * d if diag else 0
                            w = QB - n0
                            sc = psB.tile([128, 2, QB], F32, tag="sc", name=f"sc{j}_{p}_{t}")
                            for u in range(2):
                                nc.tensor.matmul(
                                    sc[:, u, 0:w],
                                    kt[jj][64 * u : 64 * u + 64, 128 * d : 128 * d + 128],
                                    qt[j][64 * u : 64 * u + 64, p, n0:QB],
                                    start=True, stop=True,
                                )
                            if diag:
                                for u in range(2):
                                    nc.vector.tensor_add(
                                        sc[:, u, 0:128], sc[:, u, 0:128], tri_sb[:]
                                    )
                            pt = ptp.tile([128, 2, QB], BF16, tag="pt")
                            nc.scalar.activation(
                                out=pt[:, :, 0:w], in_=sc[:, :, 0:w],
                                func=AF.Exp, scale=0.125,
                            )
                            for u in range(2):
                                nc.tensor.matmul(
                                    pvs[u][0:65, n0:QB],
                                    vaug[jj][:, d, :],
                                    pt[:, u, 0:w],
                                    start=(t == 0), stop=(t == ntile - 1),
                                )
                            if t % 2 == 1 or diag:
                                yield
                        for u in range(2):
                            h = 2 * p + u
                            att = t1.tile([65, QB], F32, tag="att")
                            nc.vector.tensor_copy(att[:], pvs[u][0:65, :])
                            for s in range(2):
                                shard = 2 * j + s
                                cs = slice(ROWS_PER_CORE * s, ROWS_PER_CORE * (s + 1))
                                nc.gpsimd.dma_start(
                                    a2a_in[h][shard, 0:64, :], att[0:64, cs]
                                )
                                nc.gpsimd.dma_start(
                                    a2a_in[h][shard, 64, :], att[64:65, cs]
                                )
                        yield p

                def drive(gen):
                    for _ in gen:
                        pass

                def interleave(att_gen, proj_gen, att_per_proj=1):
                    att_done = proj_done = False
                    while not (att_done and proj_done):
                        for _ in range(att_per_proj):
                            if not att_done:
                                att_done = next(att_gen, "END") == "END"
                        if not proj_done:
                            proj_done = next(proj_gen, "END") == "END"

                drive(emit_proj(0))
                # wo/sel preloads issue once the scalar engine reaches this
                # point (after proj(0)'s activations) - they would otherwise
                # compete with wq/x/rope loads for HBM at kernel start
                nc.scalar.dma_start(wo_sb[:], wo[:])
                nc.scalar.dma_start(sel_sb[:], sel16[:])
                interleave(emit_att(0), emit_proj(1), att_per_proj=1)
                interleave(emit_att(1), emit_proj(2), att_per_proj=1)
                interleave(emit_att(2), emit_proj(3), att_per_proj=1)
                for unit in emit_att(3):
                    if unit is None:
                        continue
                    for h in (2 * unit, 2 * unit + 1):
                        nc.gpsimd.collective_compute(
                            "AllToAll",
                            mybir.AluOpType.bypass,
                            replica_groups=[list(range(N_CORES))],
                            ins=[a2a_in[h][:].opt()],
                            outs=[a2a_out[h][:].opt()],
                        )

                # ============ out projection (same pools: po in acc slots,
                # bc in pv slots) ====================================
                R = ROWS_PER_CORE
                dsb_raw = [
                    t1.tile([2 * N_CORES, R], F32, tag=f"denraw{i}", name=f"denraw{i}", bufs=1)
                    for i in range(2)
                ]
                dsb_inv = [
                    t1.tile([2 * N_CORES, R], F32, tag=f"deninv{i}", name=f"deninv{i}", bufs=1)
                    for i in range(2)
                ]
                dsb = [
                    t1.tile([2 * N_CORES, R], F32R, tag=f"den{i}", name=f"den{i}", bufs=1)
                    for i in range(2)
                ]
                an_sb = pers.tile([128, 2 * N_CORES, R], BF16)

                for h in range(HPC):
                    hf, rs = h // 2, slice(8 * (h % 2), 8 * (h % 2) + 8)
                    nc.sync.dma_start(dsb_raw[hf][rs, :], a2a_out[h][:, 64, :])

                def an_half(half):
                    nc.vector.reciprocal_approx_fast(
                        out=dsb_inv[half][:, :], in_=dsb_raw[half][:, :]
                    )
                    nc.vector.tensor_copy(dsb[half][:, :], dsb_inv[half][:, :])
                    for g in range(N_CORES):
                        a_raw = t1.tile([128, R], F32, tag="araw")
                        nc.sync.dma_start(
                            a_raw[0:64, :], a2a_out[2 * half][g, 0:64, :]
                        )
                        nc.sync.dma_start(
                            a_raw[64:128, :], a2a_out[2 * half + 1][g, 0:64, :]
                        )
                        bc = psPV.tile([128, QB], F32, tag="pv", name=f"bc{half}_{g}")
                        nc.tensor.matmul(
                            bc[:, 0:R],
                            sel_sb[:, 2 * g + half, :],
                            dsb[half][:, :],
                            start=True, stop=True,
                        )
                        nc.vector.tensor_mul(
                            an_sb[:, 2 * g + half, :], a_raw[:], bc[:, 0:R]
                        )

                an_half(0)
                an_half(1)

                NB_OUT = D_IN // 512  # 4
                for nb in range(NB_OUT):
                    osl = slice(512 * nb, 512 * nb + 512)
                    po = [
                        psA.tile([128, QB], F32, tag="acc", name=f"po{q}_{nb}")
                        for q in range(2)
                    ]
                    for half in range(2):
                        for g in range(N_CORES):
                            gh = 2 * g + half
                            first = half == 0 and g == 0
                            last = half == 1 and g == N_CORES - 1
                            for q in range(2):
                                nc.tensor.matmul(
                                    po[q][:],
                                    an_sb[:, gh, 128 * q : 128 * q + 128],
                                    wo_sb[:, gh, osl],
                                    start=first, stop=last,
                                )
                    for q in range(2):
                        osb = t1.tile([128, QB], F32, tag="osb", bufs=1)
                        nc.vector.tensor_copy(osb[:], po[q][:])
                        nc.sync.dma_start(out[128 * q : 128 * q + 128, osl], osb[:])

    nc.compile()
    return nc


_NC_CACHE = None


def _get_nc():
    global _NC_CACHE
    if _NC_CACHE is None:
        _NC_CACHE = _build()
    return _NC_CACHE


def _to_ktile_layout(w):
    m = w.shape[1]
    return np.ascontiguousarray(w.reshape(KT_TILES, 128, m).transpose(1, 0, 2))


def _make_in_maps(x, cos, sin, wq, wk, wv, wo, q_norm_w, k_norm_w):
    x = np.asarray(x, dtype=np.float32)
    cos = np.asarray(cos, dtype=np.float32)
    sin = np.asarray(sin, dtype=np.float32)
    wq = np.asarray(wq, dtype=np.float32)
    wk = np.asarray(wk, dtype=np.float32)
    wv = np.asarray(wv, dtype=np.float32)
    wo = np.asarray(wo, dtype=np.float32)
    qw = np.asarray(q_norm_w, dtype=np.float32)
    kw = np.asarray(k_norm_w, dtype=np.float32)

    xT = np.ascontiguousarray(x[0].T).astype(BF16_NP)
    wo_b = _to_ktile_layout(wo).astype(BF16_NP)

    cosT = cos.T  # [64, SEQ]
    sinT = sin.T
    sgn = np.where(np.arange(64) < 32, -1.0, 1.0).astype(np.float32)
    wrot_q = qw[(np.arange(64) + 32) % 64]
    wrot_k = kw[(np.arange(64) + 32) % 64]
    cq1 = cosT * qw[:, None]
    sq1 = sinT * (sgn * wrot_q)[:, None]
    coswq = np.ascontiguousarray(np.vstack([cq1, cq1]))
    sinwq = np.ascontiguousarray(np.vstack([sq1, sq1]))
    coswk = np.ascontiguousarray(cosT * kw[:, None])
    sinwk = np.ascontiguousarray(sinT * (sgn * wrot_k)[:, None])

    ii, jj = np.meshgrid(np.arange(128), np.arange(128), indexing="ij")
    tri = np.where(ii <= jj, 0.0, NEG).astype(np.float32)
    onesblk = np.zeros((128, 128), np.float32)
    onesblk[0:64, 0:64] = 1.0
    onesblk[64:128, 64:128] = 1.0
    sel16 = np.zeros((2 * N_CORES, 2 * N_CORES, 128), np.float32)
    for g in range(N_CORES):
        for half in range(2):
            for m in range(128):
                sel16[8 * (m // 64) + g, 2 * g + half, m] = 1.0

    in_maps = []
    for c in range(N_CORES):
        wq_c = _to_ktile_layout(
            np.ascontiguousarray(wq[:, 256 * c : 256 * c + 256])
        ).astype(BF16_NP)
        wkv_c = _to_ktile_layout(
            np.ascontiguousarray(
                np.concatenate(
                    [wk[:, 64 * c : 64 * c + 64], wv[:, 64 * c : 64 * c + 64]],
                    axis=1,
                )
            )
        ).astype(BF16_NP)
        in_maps.append(
            {
                "xT": xT,
                "wq": wq_c,
                "wkv": wkv_c,
                "wo": wo_b,
                "coswq": coswq,
                "sinwq": sinwq,
                "coswk": coswk,
                "sinwk": sinwk,
                "tri": tri,
                "sel16": sel16,
                "onesblk": onesblk,
            }
        )
    return in_maps


def kernel(x, cos, sin, wq, wk, wv, wo, q_norm_w, k_norm_w):
    in_maps = _make_in_maps(x, cos, sin, wq, wk, wv, wo, q_norm_w, k_norm_w)
    nc = _get_nc()
    res = run_bass_kernel_spmd(nc, in_maps, core_ids=list(range(N_CORES)))
    rows = [res.results[c]["out"] for c in range(N_CORES)]
    full = np.concatenate(rows, axis=0)  # [SEQ, D_IN]
    return full.reshape(1, SEQ, D_IN).astype(np.float32)


# revision 17
# speedup vs baseline: 1.0950x; 1.0950x over previous
"""GQA FlashAttention (RMSNorm QK + RoPE, causal) on 8 TRN2 NeuronCores.

Sharding: tensor-parallel over heads (core c owns q-heads 4c..4c+3 and
kv-head c; the GQA group is fully local). Head-chunked AllToAlls re-shard
the attention output from head-parallel to row-parallel; each core then
computes its 256 output rows against the full Wo.

v5 vs v4:
- Row-packed scores: the K=64 scores matmuls for a pair of heads run
  concurrently in PE row-groups 0 and 64 (kt duplicated to both halves,
  qt stored pair-stacked), halving attention PE occupancy for the same
  ACT cost. This also removes the cross-partition RoPE writes.
- Two-pass projections (Q pass then KV pass over SBUF-resident x tiles)
  cut the live accumulator banks from 3 to 2, freeing PSUM for a second
  PV bank: 2 acc + 4 scores + 2 pv = 8 banks.
- Out projection runs inside the same pool scope: its half-0
  accumulation (chunks 0/1 of the AllToAll) executes in the freed acc
  banks while chunks 2/3 are still in flight.
"""

import sys

sys.path.insert(0, "/opt/trn_rl_repo")

import numpy as np
import ml_dtypes
import concourse.bass as bass  # noqa: F401
import concourse.tile as tile
from concourse import mybir, bacc
import concourse.bacc as bacc_mod
from concourse.bass_utils import run_bass_kernel_spmd
from concourse.hw_specs import get_activation_tables as _orig_get_tables
from concourse.masks import make_identity

N_CORES = 8
D_IN = 2048
SEQ = 2048
N_HEADS = 32
N_KV = 8
HD = 64
HPC = N_HEADS // N_CORES  # 4 q heads per core
EPS = 1e-6
NEG = -1.0e9

F32 = mybir.dt.float32
F32R = mybir.dt.float32r
BF16 = mybir.dt.bfloat16
BF16_NP = ml_dtypes.bfloat16

KT_TILES = D_IN // 128
QB = 512
NQB = SEQ // QB  # 4
ROWS_PER_CORE = SEQ // N_CORES  # 256
AF = mybir.ActivationFunctionType

_ONE_TABLE = "natural_log_exp_and_others"


def _pinned_tables(arch):
    tabs = _orig_get_tables(arch)
    return {n: (fs if n == _ONE_TABLE else set()) for n, fs in tabs.items()}


def _build():
    bacc_mod.get_activation_tables = _pinned_tables
    nc = bacc.Bacc(num_devices=N_CORES)

    xT = nc.dram_tensor("xT", [D_IN, SEQ], BF16, kind="ExternalInput")
    wq = nc.dram_tensor("wq", [128, KT_TILES, HPC * HD], BF16, kind="ExternalInput")
    wkv = nc.dram_tensor("wkv", [128, KT_TILES, 2 * HD], BF16, kind="ExternalInput")
    wo = nc.dram_tensor("wo", [128, KT_TILES, D_IN], BF16, kind="ExternalInput")
    coswq = nc.dram_tensor("coswq", [128, SEQ], F32, kind="ExternalInput")
    sinwq = nc.dram_tensor("sinwq", [128, SEQ], F32, kind="ExternalInput")
    coswk = nc.dram_tensor("coswk", [64, SEQ], F32, kind="ExternalInput")
    sinwk = nc.dram_tensor("sinwk", [64, SEQ], F32, kind="ExternalInput")
    tri = nc.dram_tensor("tri", [128, 128], F32, kind="ExternalInput")
    sel16 = nc.dram_tensor("sel16", [2 * N_CORES, 2 * N_CORES, 128], F32R, kind="ExternalInput")
    onesblk_in = nc.dram_tensor("onesblk", [128, 128], F32R, kind="ExternalInput")

    out = nc.dram_tensor("out", [ROWS_PER_CORE, D_IN], F32, kind="ExternalOutput")

    with tile.TileContext(nc) as tc:
        with (
            tc.tile_pool(name="persist", bufs=1) as pers,
            tc.tile_pool(name="dram", bufs=1, space="DRAM") as dram,
        ):
            # ---- persistent SBUF preloads (contiguous, host-transposed) ----
            wq_sb = pers.tile([128, KT_TILES, HPC * HD], BF16)
            nc.sync.dma_start(wq_sb[:], wq[:])
            wkv_sb = pers.tile([128, KT_TILES, 2 * HD], BF16)
            nc.sync.dma_start(wkv_sb[:], wkv[:])

            cq_sb = pers.tile([128, SEQ], F32)
            sq_sb = pers.tile([128, SEQ], F32)
            ck_sb = pers.tile([64, SEQ], F32)
            sk_sb = pers.tile([64, SEQ], F32)
            nc.gpsimd.dma_start(cq_sb[:], coswq[:])
            nc.gpsimd.dma_start(sq_sb[:], sinwq[:])
            nc.gpsimd.dma_start(ck_sb[:], coswk[:])
            nc.gpsimd.dma_start(sk_sb[:], sinwk[:])
            tri_sb = pers.tile([128, 128], F32)
            nc.gpsimd.dma_start(tri_sb[:], tri[:])
            onesblk = pers.tile([128, 128], F32R)
            nc.gpsimd.dma_start(onesblk[:], onesblk_in[:])

            wo_sb = pers.tile([128, KT_TILES, D_IN], BF16)  # 8 MB
            sel_sb = pers.tile([2 * N_CORES, 2 * N_CORES, 128], F32R)

            ident = pers.tile([128, 128], F32)
            make_identity(nc, ident[:])
            eps_sb = pers.tile([128, 1], F32)
            nc.vector.memset(eps_sb[:], EPS)

            # pair-stacked q (pair p holds heads 2p/2p+1 in partition halves)
            qt = [pers.tile([128, 2, QB], BF16, name=f"qt{j}") for j in range(NQB)]
            # k duplicated into both partition halves for row-group packing
            kt = [pers.tile([128, QB], BF16, name=f"kt{j}") for j in range(NQB)]
            vaug = [pers.tile([128, 4, HD + 1], BF16, name=f"va{j}") for j in range(NQB)]

            a2a_in = [
                dram.tile([N_CORES, HD + 1, ROWS_PER_CORE], F32, name=f"a2ai{h}")
                for h in range(HPC)
            ]
            a2a_out = [
                dram.tile([N_CORES, HD + 1, ROWS_PER_CORE], F32, name=f"a2ao{h}")
                for h in range(HPC)
            ]
            cc_warm_in = dram.tile([N_CORES, 4], F32, name="ccwi")
            cc_warm_out = dram.tile([N_CORES, 4], F32, name="ccwo")

            # ============ fused projections + attention ====================
            with (
                tc.tile_pool(name="xt", bufs=18) as xp,
                tc.tile_pool(name="acc", bufs=2, space="PSUM") as psA,
                tc.tile_pool(name="sc", bufs=2, space="PSUM") as psB,
                tc.tile_pool(name="pv", bufs=2, space="PSUM") as psPV,
                tc.tile_pool(name="work", bufs=2) as t1,
                tc.tile_pool(name="ptp", bufs=3) as ptp,
            ):

                def norm_rope(j, raw_psum, idx):
                    """Evict + rmsnorm + rope one accumulator. idx 0/1 = q
                    pairs, idx 2 = kv. Generator (yields mid-chain)."""
                    sl = slice(QB * j, QB * j + QB)
                    is_kv = idx == 2
                    rows = slice(0, 64) if is_kv else slice(0, 128)
                    rawsb = t1.tile([128, QB], F32, tag="rawsb")
                    nc.vector.tensor_copy(rawsb[:], raw_psum[:])
                    sq = t1.tile([128, QB], F32R, tag="sq")
                    nc.vector.tensor_mul(sq[:], rawsb[:], rawsb[:])
                    psn = psB.tile([128, 2, QB], F32, tag="sc", name=f"psn{idx}_{j}")
                    nc.tensor.matmul(psn[:, 0, :], onesblk[:], sq[:], start=True, stop=True)
                    lnv = t1.tile([128, QB], F32, tag="lnv", bufs=1)
                    nc.scalar.activation(
                        out=lnv[rows, :], in_=psn[rows, 0, :],
                        func=AF.Ln, bias=eps_sb[rows, :], scale=1.0 / HD,
                    )
                    rcp = t1.tile([128, QB], F32, tag="rcp", bufs=1)
                    nc.scalar.activation(
                        out=rcp[rows, :], in_=lnv[rows, :], func=AF.Exp, scale=-0.5,
                    )
                    yield
                    tn = t1.tile([128, QB], F32, tag="tn")
                    nc.vector.tensor_mul(tn[rows, :], rawsb[rows, :], rcp[rows, :])
                    rot = t1.tile([128, QB], F32, tag="rot")
                    nh = 1 if is_kv else 2
                    for b in range(nh):
                        o = 64 * b
                        nc.vector.tensor_copy(rot[o : o + 32, :], tn[o + 32 : o + 64, :])
                        nc.vector.tensor_copy(rot[o + 32 : o + 64, :], tn[o : o + 32, :])
                    if is_kv:
                        tcs = t1.tile([64, QB], F32, tag="tcs", bufs=1)
                        nc.vector.tensor_mul(tcs[:], tn[0:64, :], ck_sb[:, sl])
                        nc.vector.tensor_mul(rot[0:64, :], rot[0:64, :], sk_sb[:, sl])
                        nc.vector.tensor_add(kt[j][0:64, :], tcs[:], rot[0:64, :])
                        nc.vector.tensor_copy(kt[j][64:128, :], kt[j][0:64, :])
                        vt = t1.tile([64, QB], F32, tag="vt", bufs=1)
                        nc.vector.tensor_copy(vt[:], rawsb[64:128, :])
                        for d in range(4):
                            psv = psB.tile([128, 2, QB], F32, tag="sc", name=f"psv{j}_{d}")
                            nc.tensor.transpose(
                                psv[:, 0, 0:64],
                                vt[:, 128 * d : 128 * d + 128],
                                ident[0:64, 0:64],
                            )
                            nc.vector.tensor_copy(vaug[j][:, d, 0:HD], psv[:, 0, 0:64])
                            nc.vector.memset(vaug[j][:, d, HD : HD + 1], 1.0)
                            if d == 1:
                                yield
                    else:
                        tc2 = t1.tile([128, QB], F32, tag="tc2")
                        nc.vector.tensor_mul(tc2[:], tn[:], cq_sb[:, sl])
                        nc.vector.tensor_mul(rot[:], rot[:], sq_sb[:, sl])
                        nc.vector.tensor_add(qt[j][:, idx, :], tc2[:], rot[:])
                    yield

                def emit_proj(j):
                    """Two-pass projection: Q (2 banks) then KV (1 bank),
                    sharing one SBUF-resident set of x tiles."""
                    sl = slice(QB * j, QB * j + QB)
                    xts = []
                    accq = [
                        psA.tile([128, QB], F32, tag="acc", name=f"accq{i}_{j}")
                        for i in range(2)
                    ]
                    for k in range(KT_TILES):
                        xt = xp.tile([128, QB], BF16, tag="xt", name=f"xt{j}_{k}")
                        xts.append(xt)
                        nc.sync.dma_start(xt[:], xT[128 * k : 128 * k + 128, sl])
                        st = k == 0
                        sp = k == KT_TILES - 1
                        nc.tensor.matmul(accq[0][:], wq_sb[:, k, 0:128], xt[:], start=st, stop=sp)
                        nc.tensor.matmul(accq[1][:], wq_sb[:, k, 128:256], xt[:], start=st, stop=sp)
                        if k % 2 == 1:
                            yield
                    yield from norm_rope(j, accq[0], 0)
                    yield from norm_rope(j, accq[1], 1)
                    acckv = psA.tile([128, QB], F32, tag="acc", name=f"acckv_{j}")
                    for k in range(KT_TILES):
                        st = k == 0
                        sp = k == KT_TILES - 1
                        nc.tensor.matmul(acckv[:], wkv_sb[:, k, :], xts[k][:], start=st, stop=sp)
                        if k % 4 == 3:
                            yield
                    yield from norm_rope(j, acckv, 2)

                def emit_att(j):
                    """Attention for block j, head-pair packed: scores for
                    heads 2p/2p+1 run concurrently in PE row groups 0/64.
                    Yields after each unit; yields (pair index) after a
                    pair's output is fully stored."""
                    for p in range(2):
                        pvs = [
                            psPV.tile([128, QB], F32, tag="pv", name=f"pv{j}_{p}_{u}")
                            for u in range(2)
                        ]
                        ntile = 4 * j + 4
                        for t in range(ntile):
                            jj, d = t // 4, t % 4
                            diag = jj == j
                            n0 = 128 * d if diag else 0
                            w = QB - n0
                            sc = psB.tile([128, 2, QB], F32, tag="sc", name=f"sc{j}_{p}_{t}")
                            for u in range(2):
                                nc.tensor.matmul(
                                    sc[:, u, 0:w],
                                    kt[jj][64 * u : 64 * u + 64, 128 * d : 128 * d + 128],
                                    qt[j][64 * u : 64 * u + 64, p, n0:QB],
                                    start=True, stop=True,
                                )
                            if diag:
                                for u in range(2):
                                    nc.vector.tensor_add(
                                        sc[:, u, 0:128], sc[:, u, 0:128], tri_sb[:]
                                    )
                            pt = ptp.tile([128, 2, QB], BF16, tag="pt")
                            nc.scalar.activation(
                                out=pt[:, :, 0:w], in_=sc[:, :, 0:w],
                                func=AF.Exp, scale=0.125,
                            )
                            for u in range(2):
                                nc.tensor.matmul(
                                    pvs[u][0:65, n0:QB],
                                    vaug[jj][:, d, :],
                                    pt[:, u, 0:w],
                                    start=(t == 0), stop=(t == ntile - 1),
                                )
                            if t % 2 == 1 or diag:
                                yield
                        for u in range(2):
                            h = 2 * p + u
                            att = t1.tile([65, QB], F32, tag="att")
                            nc.vector.tensor_copy(att[:], pvs[u][0:65, :])
                            for s in range(2):
                                shard = 2 * j + s
                                cs = slice(ROWS_PER_CORE * s, ROWS_PER_CORE * (s + 1))
                                nc.gpsimd.dma_start(
                                    a2a_in[h][shard, 0:64, :], att[0:64, cs]
                                )
                                nc.gpsimd.dma_start(
                                    a2a_in[h][shard, 64, :], att[64:65, cs]
                                )
                        yield p

                def drive(gen):
                    for _ in gen:
                        pass

                def interleave(att_gen, proj_gen, att_per_proj=1):
                    att_done = proj_done = False
                    while not (att_done and proj_done):
                        for _ in range(att_per_proj):
                            if not att_done:
                                att_done = next(att_gen, "END") == "END"
                        if not proj_done:
                            proj_done = next(proj_gen, "END") == "END"

                # warmup collective: the first collective after load pays
                # ~25-30us of one-time ncfw setup; prepay it during compute
                nc.gpsimd.collective_compute(
                    "AllToAll",
                    mybir.AluOpType.bypass,
                    replica_groups=[list(range(N_CORES))],
                    ins=[cc_warm_in[:].opt()],
                    outs=[cc_warm_out[:].opt()],
                )

                drive(emit_proj(0))
                # wo/sel preloads issue once the scalar engine reaches this
                # point (after proj(0)'s activations) - they would otherwise
                # compete with wq/x/rope loads for HBM at kernel start
                nc.scalar.dma_start(wo_sb[:], wo[:])
                nc.scalar.dma_start(sel_sb[:], sel16[:])
                interleave(emit_att(0), emit_proj(1), att_per_proj=1)
                interleave(emit_att(1), emit_proj(2), att_per_proj=1)
                interleave(emit_att(2), emit_proj(3), att_per_proj=1)

                # ---- last block's attention + pipelined reshard/out-proj ----
                R = ROWS_PER_CORE
                dsb_raw = [
                    t1.tile([2 * N_CORES, R], F32, tag=f"denraw{i}", name=f"denraw{i}", bufs=1)
                    for i in range(2)
                ]
                dsb_inv = [
                    t1.tile([2 * N_CORES, R], F32, tag=f"deninv{i}", name=f"deninv{i}", bufs=1)
                    for i in range(2)
                ]
                dsb = [
                    t1.tile([2 * N_CORES, R], F32R, tag=f"den{i}", name=f"den{i}", bufs=1)
                    for i in range(2)
                ]
                an_sb = pers.tile([128, 2 * N_CORES, R], BF16)

                def emit_chunk(h):
                    nc.gpsimd.collective_compute(
                        "AllToAll",
                        mybir.AluOpType.bypass,
                        replica_groups=[list(range(N_CORES))],
                        ins=[a2a_in[h][:].opt()],
                        outs=[a2a_out[h][:].opt()],
                    )

                def emit_den(h):
                    hf, rs = h // 2, slice(8 * (h % 2), 8 * (h % 2) + 8)
                    nc.sync.dma_start(dsb_raw[hf][rs, :], a2a_out[h][:, 64, :])

                def an_half(half, bc_pool, bc_tag):
                    """Generator: normalize the attnT slices for chunk pair
                    `half` (one unit per source core)."""
                    nc.vector.reciprocal_approx_fast(
                        out=dsb_inv[half][:, :], in_=dsb_raw[half][:, :]
                    )
                    nc.vector.tensor_copy(dsb[half][:, :], dsb_inv[half][:, :])
                    for g in range(N_CORES):
                        a_raw = t1.tile([128, R], F32, tag="araw")
                        nc.sync.dma_start(
                            a_raw[0:64, :], a2a_out[2 * half][g, 0:64, :]
                        )
                        nc.sync.dma_start(
                            a_raw[64:128, :], a2a_out[2 * half + 1][g, 0:64, :]
                        )
                        bc = bc_pool.tile(
                            [128, QB], F32, tag=bc_tag, name=f"bc{half}_{g}"
                        )
                        nc.tensor.matmul(
                            bc[:, 0:R],
                            sel_sb[:, 2 * g + half, :],
                            dsb[half][:, :],
                            start=True, stop=True,
                        )
                        nc.vector.tensor_mul(
                            an_sb[:, 2 * g + half, :], a_raw[:], bc[:, 0:R]
                        )
                        yield

                att3 = emit_att(3)
                for unit in att3:
                    if unit == 0:
                        break
                # pair 0 stored: fire chunks 0,1 and prep half-0 an slices
                # (bc matmuls borrow the acc banks, idle with no proj left)
                emit_chunk(0)
                emit_chunk(1)
                emit_den(0)
                emit_den(1)
                an0 = an_half(0, psA, "acc")
                interleave(att3, an0, att_per_proj=2)
                drive(an0)
                emit_chunk(2)
                emit_chunk(3)
                emit_den(2)
                emit_den(3)

                # out projection in two nb-pair passes over acc+pv banks;
                # half-0 accumulation of pass 1 overlaps chunks 2/3
                def po_pass(nbs, emit_an1):
                    po = {}
                    for i, nb in enumerate(nbs):
                        po[nb] = [
                            (psA if i == 0 else psPV).tile(
                                [128, QB], F32,
                                tag=("acc" if i == 0 else "pv"),
                                name=f"po{q}_{nb}",
                            )
                            for q in range(2)
                        ]
                    for half in range(2):
                        for g in range(N_CORES):
                            gh = 2 * g + half
                            first = half == 0 and g == 0
                            last = half == 1 and g == N_CORES - 1
                            for nb in nbs:
                                for q in range(2):
                                    nc.tensor.matmul(
                                        po[nb][q][:],
                                        an_sb[:, gh, 128 * q : 128 * q + 128],
                                        wo_sb[:, gh, 512 * nb : 512 * nb + 512],
                                        start=first, stop=last,
                                    )
                        if half == 0 and emit_an1:
                            drive(an_half(1, psB, "sc"))
                    for nb in nbs:
                        for q in range(2):
                            osb = t1.tile([128, QB], F32, tag="osb")
                            nc.vector.tensor_copy(osb[:], po[nb][q][:])
                            nc.sync.dma_start(
                                out[128 * q : 128 * q + 128, 512 * nb : 512 * nb + 512],
                                osb[:],
                            )

                po_pass((0, 1), emit_an1=True)
                po_pass((2, 3), emit_an1=False)

    nc.compile()
    return nc


_NC_CACHE = None


def _get_nc():
    global _NC_CACHE
    if _NC_CACHE is None:
        _NC_CACHE = _build()
    return _NC_CACHE


def _to_ktile_layout(w):
    m = w.shape[1]
    return np.ascontiguousarray(w.reshape(KT_TILES, 128, m).transpose(1, 0, 2))


def _make_in_maps(x, cos, sin, wq, wk, wv, wo, q_norm_w, k_norm_w):
    x = np.asarray(x, dtype=np.float32)
    cos = np.asarray(cos, dtype=np.float32)
    sin = np.asarray(sin, dtype=np.float32)
    wq = np.asarray(wq, dtype=np.float32)
    wk = np.asarray(wk, dtype=np.float32)
    wv = np.asarray(wv, dtype=np.float32)
    wo = np.asarray(wo, dtype=np.float32)
    qw = np.asarray(q_norm_w, dtype=np.float32)
    kw = np.asarray(k_norm_w, dtype=np.float32)

    xT = np.ascontiguousarray(x[0].T).astype(BF16_NP)
    wo_b = _to_ktile_layout(wo).astype(BF16_NP)

    cosT = cos.T  # [64, SEQ]
    sinT = sin.T
    sgn = np.where(np.arange(64) < 32, -1.0, 1.0).astype(np.float32)
    wrot_q = qw[(np.arange(64) + 32) % 64]
    wrot_k = kw[(np.arange(64) + 32) % 64]
    cq1 = cosT * qw[:, None]
    sq1 = sinT * (sgn * wrot_q)[:, None]
    coswq = np.ascontiguousarray(np.vstack([cq1, cq1]))
    sinwq = np.ascontiguousarray(np.vstack([sq1, sq1]))
    coswk = np.ascontiguousarray(cosT * kw[:, None])
    sinwk = np.ascontiguousarray(sinT * (sgn * wrot_k)[:, None])

    ii, jj = np.meshgrid(np.arange(128), np.arange(128), indexing="ij")
    tri = np.where(ii <= jj, 0.0, NEG).astype(np.float32)
    onesblk = np.zeros((128, 128), np.float32)
    onesblk[0:64, 0:64] = 1.0
    onesblk[64:128, 64:128] = 1.0
    sel16 = np.zeros((2 * N_CORES, 2 * N_CORES, 128), np.float32)
    for g in range(N_CORES):
        for half in range(2):
            for m in range(128):
                sel16[8 * (m // 64) + g, 2 * g + half, m] = 1.0

    in_maps = []
    for c in range(N_CORES):
        wq_c = _to_ktile_layout(
            np.ascontiguousarray(wq[:, 256 * c : 256 * c + 256])
        ).astype(BF16_NP)
        wkv_c = _to_ktile_layout(
            np.ascontiguousarray(
                np.concatenate(
                    [wk[:, 64 * c : 64 * c + 64], wv[:, 64 * c : 64 * c + 64]],
                    axis=1,
                )
            )
        ).astype(BF16_NP)
        in_maps.append(
            {
                "xT": xT,
                "wq": wq_c,
                "wkv": wkv_c,
                "wo": wo_b,
                "coswq": coswq,
                "sinwq": sinwq,
                "coswk": coswk,
                "sinwk": sinwk,
                "tri": tri,
                "sel16": sel16,
                "onesblk": onesblk,
            }
        )
    return in_maps


def kernel(x, cos, sin, wq, wk, wv, wo, q_norm_w, k_norm_w):
    in_maps = _make_in_maps(x, cos, sin, wq, wk, wv, wo, q_norm_w, k_norm_w)
    nc = _get_nc()
    res = run_bass_kernel_spmd(nc, in_maps, core_ids=list(range(N_CORES)))
    rows = [res.results[c]["out"] for c in range(N_CORES)]
    full = np.concatenate(rows, axis=0)  # [SEQ, D_IN]
    return full.reshape(1, SEQ, D_IN).astype(np.float32)


# revision 18
# speedup vs baseline: 1.1137x; 1.0171x over previous
"""GQA FlashAttention (RMSNorm QK + RoPE, causal) on 8 TRN2 NeuronCores.

Sharding: tensor-parallel over heads (core c owns q-heads 4c..4c+3 and
kv-head c; the GQA group is fully local). Head-chunked AllToAlls re-shard
the attention output from head-parallel to row-parallel; each core then
computes its 256 output rows against the full Wo.

v5 vs v4:
- Row-packed scores: the K=64 scores matmuls for a pair of heads run
  concurrently in PE row-groups 0 and 64 (kt duplicated to both halves,
  qt stored pair-stacked), halving attention PE occupancy for the same
  ACT cost. This also removes the cross-partition RoPE writes.
- Two-pass projections (Q pass then KV pass over SBUF-resident x tiles)
  cut the live accumulator banks from 3 to 2, freeing PSUM for a second
  PV bank: 2 acc + 4 scores + 2 pv = 8 banks.
- Out projection runs inside the same pool scope: its half-0
  accumulation (chunks 0/1 of the AllToAll) executes in the freed acc
  banks while chunks 2/3 are still in flight.
"""

import sys

sys.path.insert(0, "/opt/trn_rl_repo")

import numpy as np
import ml_dtypes
import concourse.bass as bass  # noqa: F401
import concourse.tile as tile
from concourse import mybir, bacc
import concourse.bacc as bacc_mod
from concourse.bass_utils import run_bass_kernel_spmd
from concourse.hw_specs import get_activation_tables as _orig_get_tables
from concourse.masks import make_identity

N_CORES = 8
D_IN = 2048
SEQ = 2048
N_HEADS = 32
N_KV = 8
HD = 64
HPC = N_HEADS // N_CORES  # 4 q heads per core
EPS = 1e-6
NEG = -1.0e9

F32 = mybir.dt.float32
F32R = mybir.dt.float32r
BF16 = mybir.dt.bfloat16
BF16_NP = ml_dtypes.bfloat16

KT_TILES = D_IN // 128
QB = 512
NQB = SEQ // QB  # 4
ROWS_PER_CORE = SEQ // N_CORES  # 256
AF = mybir.ActivationFunctionType

_ONE_TABLE = "natural_log_exp_and_others"


def _pinned_tables(arch):
    tabs = _orig_get_tables(arch)
    return {n: (fs if n == _ONE_TABLE else set()) for n, fs in tabs.items()}


def _build():
    bacc_mod.get_activation_tables = _pinned_tables
    nc = bacc.Bacc(num_devices=N_CORES)

    xT = nc.dram_tensor("xT", [D_IN, SEQ], BF16, kind="ExternalInput")
    wq = nc.dram_tensor("wq", [128, KT_TILES, HPC * HD], BF16, kind="ExternalInput")
    wkv = nc.dram_tensor("wkv", [128, KT_TILES, 2 * HD], BF16, kind="ExternalInput")
    wo = nc.dram_tensor("wo", [128, KT_TILES, D_IN], BF16, kind="ExternalInput")
    coswq = nc.dram_tensor("coswq", [128, SEQ], F32, kind="ExternalInput")
    sinwq = nc.dram_tensor("sinwq", [128, SEQ], F32, kind="ExternalInput")
    coswk = nc.dram_tensor("coswk", [64, SEQ], F32, kind="ExternalInput")
    sinwk = nc.dram_tensor("sinwk", [64, SEQ], F32, kind="ExternalInput")
    tri = nc.dram_tensor("tri", [128, 128], F32, kind="ExternalInput")
    sel16 = nc.dram_tensor("sel16", [2 * N_CORES, 2 * N_CORES, 128], F32R, kind="ExternalInput")
    onesblk_in = nc.dram_tensor("onesblk", [128, 128], F32R, kind="ExternalInput")

    out = nc.dram_tensor("out", [ROWS_PER_CORE, D_IN], F32, kind="ExternalOutput")

    with tile.TileContext(nc) as tc:
        with (
            tc.tile_pool(name="persist", bufs=1) as pers,
            tc.tile_pool(name="dram", bufs=1, space="DRAM") as dram,
        ):
            # ---- persistent SBUF preloads (contiguous, host-transposed) ----
            wq_sb = pers.tile([128, KT_TILES, HPC * HD], BF16)
            wkv_sb = pers.tile([128, KT_TILES, 2 * HD], BF16)
            # per-ktile slices: the k=0 matmul only waits on 64KB, so the
            # PE starts ~2us in instead of behind the whole preload
            for k in range(KT_TILES):
                nc.sync.dma_start(wq_sb[:, k, :], wq[:, k, :])
                nc.sync.dma_start(wkv_sb[:, k, :], wkv[:, k, :])

            cq_sb = pers.tile([128, SEQ], F32)
            sq_sb = pers.tile([128, SEQ], F32)
            ck_sb = pers.tile([64, SEQ], F32)
            sk_sb = pers.tile([64, SEQ], F32)
            nc.gpsimd.dma_start(cq_sb[:], coswq[:])
            nc.gpsimd.dma_start(sq_sb[:], sinwq[:])
            nc.gpsimd.dma_start(ck_sb[:], coswk[:])
            nc.gpsimd.dma_start(sk_sb[:], sinwk[:])
            tri_sb = pers.tile([128, 128], F32)
            nc.gpsimd.dma_start(tri_sb[:], tri[:])
            onesblk = pers.tile([128, 128], F32R)
            nc.gpsimd.dma_start(onesblk[:], onesblk_in[:])

            wo_sb = pers.tile([128, KT_TILES, D_IN], BF16)  # 8 MB
            sel_sb = pers.tile([2 * N_CORES, 2 * N_CORES, 128], F32R)

            ident = pers.tile([128, 128], F32)
            make_identity(nc, ident[:])
            eps_sb = pers.tile([128, 1], F32)
            nc.vector.memset(eps_sb[:], EPS)

            # pair-stacked q (pair p holds heads 2p/2p+1 in partition halves)
            qt = [pers.tile([128, 2, QB], BF16, name=f"qt{j}") for j in range(NQB)]
            # k duplicated into both partition halves for row-group packing
            kt = [pers.tile([128, QB], BF16, name=f"kt{j}") for j in range(NQB)]
            vaug = [pers.tile([128, 4, HD + 1], BF16, name=f"va{j}") for j in range(NQB)]

            a2a_in = [
                dram.tile([N_CORES, HD + 1, ROWS_PER_CORE], F32, name=f"a2ai{h}")
                for h in range(HPC)
            ]
            a2a_out = [
                dram.tile([N_CORES, HD + 1, ROWS_PER_CORE], F32, name=f"a2ao{h}")
                for h in range(HPC)
            ]
            cc_warm_in = dram.tile([N_CORES, 4], F32, name="ccwi")
            cc_warm_out = dram.tile([N_CORES, 4], F32, name="ccwo")

            # ============ fused projections + attention ====================
            with (
                tc.tile_pool(name="xt", bufs=18) as xp,
                tc.tile_pool(name="acc", bufs=2, space="PSUM") as psA,
                tc.tile_pool(name="sc", bufs=2, space="PSUM") as psB,
                tc.tile_pool(name="pv", bufs=2, space="PSUM") as psPV,
                tc.tile_pool(name="work", bufs=2) as t1,
                tc.tile_pool(name="ptp", bufs=3) as ptp,
            ):

                def norm_rope(j, raw_psum, idx):
                    """Evict + rmsnorm + rope one accumulator. idx 0/1 = q
                    pairs, idx 2 = kv. Generator (yields mid-chain)."""
                    sl = slice(QB * j, QB * j + QB)
                    is_kv = idx == 2
                    rows = slice(0, 64) if is_kv else slice(0, 128)
                    rawsb = t1.tile([128, QB], F32, tag="rawsb")
                    nc.vector.tensor_copy(rawsb[:], raw_psum[:])
                    sq = t1.tile([128, QB], F32R, tag="sq")
                    nc.vector.tensor_mul(sq[:], rawsb[:], rawsb[:])
                    psn = psB.tile([128, 2, QB], F32, tag="sc", name=f"psn{idx}_{j}")
                    nc.tensor.matmul(psn[:, 0, :], onesblk[:], sq[:], start=True, stop=True)
                    lnv = t1.tile([128, QB], F32, tag="lnv", bufs=1)
                    nc.scalar.activation(
                        out=lnv[rows, :], in_=psn[rows, 0, :],
                        func=AF.Ln, bias=eps_sb[rows, :], scale=1.0 / HD,
                    )
                    rcp = t1.tile([128, QB], F32, tag="rcp", bufs=1)
                    nc.scalar.activation(
                        out=rcp[rows, :], in_=lnv[rows, :], func=AF.Exp, scale=-0.5,
                    )
                    yield
                    tn = t1.tile([128, QB], F32, tag="tn")
                    nc.vector.tensor_mul(tn[rows, :], rawsb[rows, :], rcp[rows, :])
                    rot = t1.tile([128, QB], F32, tag="rot")
                    nh = 1 if is_kv else 2
                    for b in range(nh):
                        o = 64 * b
                        nc.vector.tensor_copy(rot[o : o + 32, :], tn[o + 32 : o + 64, :])
                        nc.vector.tensor_copy(rot[o + 32 : o + 64, :], tn[o : o + 32, :])
                    if is_kv:
                        tcs = t1.tile([64, QB], F32, tag="tcs", bufs=1)
                        nc.vector.tensor_mul(tcs[:], tn[0:64, :], ck_sb[:, sl])
                        nc.vector.tensor_mul(rot[0:64, :], rot[0:64, :], sk_sb[:, sl])
                        nc.vector.tensor_add(kt[j][0:64, :], tcs[:], rot[0:64, :])
                        nc.vector.tensor_copy(kt[j][64:128, :], kt[j][0:64, :])
                        vt = t1.tile([64, QB], F32, tag="vt", bufs=1)
                        nc.vector.tensor_copy(vt[:], rawsb[64:128, :])
                        for d in range(4):
                            psv = psB.tile([128, 2, QB], F32, tag="sc", name=f"psv{j}_{d}")
                            nc.tensor.transpose(
                                psv[:, 0, 0:64],
                                vt[:, 128 * d : 128 * d + 128],
                                ident[0:64, 0:64],
                            )
                            nc.vector.tensor_copy(vaug[j][:, d, 0:HD], psv[:, 0, 0:64])
                            nc.vector.memset(vaug[j][:, d, HD : HD + 1], 1.0)
                            if d == 1:
                                yield
                    else:
                        tc2 = t1.tile([128, QB], F32, tag="tc2")
                        nc.vector.tensor_mul(tc2[:], tn[:], cq_sb[:, sl])
                        nc.vector.tensor_mul(rot[:], rot[:], sq_sb[:, sl])
                        nc.vector.tensor_add(qt[j][:, idx, :], tc2[:], rot[:])
                    yield

                def emit_proj(j):
                    """Two-pass projection: Q (2 banks) then KV (1 bank),
                    sharing one SBUF-resident set of x tiles."""
                    sl = slice(QB * j, QB * j + QB)
                    xts = []
                    accq = [
                        psA.tile([128, QB], F32, tag="acc", name=f"accq{i}_{j}")
                        for i in range(2)
                    ]
                    for k in range(KT_TILES):
                        xt = xp.tile([128, QB], BF16, tag="xt", name=f"xt{j}_{k}")
                        xts.append(xt)
                        nc.sync.dma_start(xt[:], xT[128 * k : 128 * k + 128, sl])
                        st = k == 0
                        sp = k == KT_TILES - 1
                        nc.tensor.matmul(accq[0][:], wq_sb[:, k, 0:128], xt[:], start=st, stop=sp)
                        nc.tensor.matmul(accq[1][:], wq_sb[:, k, 128:256], xt[:], start=st, stop=sp)
                        if k % 2 == 1:
                            yield
                    yield from norm_rope(j, accq[0], 0)
                    yield from norm_rope(j, accq[1], 1)
                    acckv = psA.tile([128, QB], F32, tag="acc", name=f"acckv_{j}")
                    for k in range(KT_TILES):
                        st = k == 0
                        sp = k == KT_TILES - 1
                        nc.tensor.matmul(acckv[:], wkv_sb[:, k, :], xts[k][:], start=st, stop=sp)
                        if k % 4 == 3:
                            yield
                    yield from norm_rope(j, acckv, 2)

                def emit_att(j):
                    """Attention for block j, head-pair packed: scores for
                    heads 2p/2p+1 run concurrently in PE row groups 0/64.
                    Yields after each unit; yields (pair index) after a
                    pair's output is fully stored."""
                    for p in range(2):
                        pvs = [
                            psPV.tile([128, QB], F32, tag="pv", name=f"pv{j}_{p}_{u}")
                            for u in range(2)
                        ]
                        ntile = 4 * j + 4
                        for t in range(ntile):
                            jj, d = t // 4, t % 4
                            diag = jj == j
                            n0 = 128 * d if diag else 0
                            w = QB - n0
                            sc = psB.tile([128, 2, QB], F32, tag="sc", name=f"sc{j}_{p}_{t}")
                            for u in range(2):
                                nc.tensor.matmul(
                                    sc[:, u, 0:w],
                                    kt[jj][64 * u : 64 * u + 64, 128 * d : 128 * d + 128],
                                    qt[j][64 * u : 64 * u + 64, p, n0:QB],
                                    start=True, stop=True,
                                )
                            if diag:
                                for u in range(2):
                                    nc.vector.tensor_add(
                                        sc[:, u, 0:128], sc[:, u, 0:128], tri_sb[:]
                                    )
                            pt = ptp.tile([128, 2, QB], BF16, tag="pt")
                            nc.scalar.activation(
                                out=pt[:, :, 0:w], in_=sc[:, :, 0:w],
                                func=AF.Exp, scale=0.125,
                            )
                            for u in range(2):
                                nc.tensor.matmul(
                                    pvs[u][0:65, n0:QB],
                                    vaug[jj][:, d, :],
                                    pt[:, u, 0:w],
                                    start=(t == 0), stop=(t == ntile - 1),
                                )
                            if t % 2 == 1 or diag:
                                yield
                        for u in range(2):
                            h = 2 * p + u
                            att = t1.tile([65, QB], F32, tag="att")
                            nc.vector.tensor_copy(att[:], pvs[u][0:65, :])
                            for s in range(2):
                                shard = 2 * j + s
                                cs = slice(ROWS_PER_CORE * s, ROWS_PER_CORE * (s + 1))
                                nc.gpsimd.dma_start(
                                    a2a_in[h][shard, 0:64, :], att[0:64, cs]
                                )
                                nc.gpsimd.dma_start(
                                    a2a_in[h][shard, 64, :], att[64:65, cs]
                                )
                        yield p

                def drive(gen):
                    for _ in gen:
                        pass

                def interleave(att_gen, proj_gen, att_per_proj=1):
                    att_done = proj_done = False
                    while not (att_done and proj_done):
                        for _ in range(att_per_proj):
                            if not att_done:
                                att_done = next(att_gen, "END") == "END"
                        if not proj_done:
                            proj_done = next(proj_gen, "END") == "END"

                # warmup collective: the first collective after load pays
                # ~25-30us of one-time ncfw setup; prepay it during compute
                nc.gpsimd.collective_compute(
                    "AllToAll",
                    mybir.AluOpType.bypass,
                    replica_groups=[list(range(N_CORES))],
                    ins=[cc_warm_in[:].opt()],
                    outs=[cc_warm_out[:].opt()],
                )

                drive(emit_proj(0))
                nc.scalar.dma_start(sel_sb[:], sel16[:])
                interleave(emit_att(0), emit_proj(1), att_per_proj=1)
                # wo preload issues from the gpsimd queue after att(0)'s
                # stores - late enough not to steal HBM bandwidth from the
                # first blocks, early enough to land before out-projection
                nc.gpsimd.dma_start(wo_sb[:], wo[:])
                interleave(emit_att(1), emit_proj(2), att_per_proj=1)
                interleave(emit_att(2), emit_proj(3), att_per_proj=1)

                # ---- last block's attention + pipelined reshard/out-proj ----
                R = ROWS_PER_CORE
                dsb_raw = [
                    t1.tile([2 * N_CORES, R], F32, tag=f"denraw{i}", name=f"denraw{i}", bufs=1)
                    for i in range(2)
                ]
                dsb_inv = [
                    t1.tile([2 * N_CORES, R], F32, tag=f"deninv{i}", name=f"deninv{i}", bufs=1)
                    for i in range(2)
                ]
                dsb = [
                    t1.tile([2 * N_CORES, R], F32R, tag=f"den{i}", name=f"den{i}", bufs=1)
                    for i in range(2)
                ]
                an_sb = pers.tile([128, 2 * N_CORES, R], BF16)

                def emit_chunk(h):
                    nc.gpsimd.collective_compute(
                        "AllToAll",
                        mybir.AluOpType.bypass,
                        replica_groups=[list(range(N_CORES))],
                        ins=[a2a_in[h][:].opt()],
                        outs=[a2a_out[h][:].opt()],
                    )

                def emit_den(h):
                    hf, rs = h // 2, slice(8 * (h % 2), 8 * (h % 2) + 8)
                    nc.sync.dma_start(dsb_raw[hf][rs, :], a2a_out[h][:, 64, :])

                def an_half(half, bc_pool, bc_tag):
                    """Generator: normalize the attnT slices for chunk pair
                    `half` (one unit per source core)."""
                    nc.vector.reciprocal_approx_fast(
                        out=dsb_inv[half][:, :], in_=dsb_raw[half][:, :]
                    )
                    nc.vector.tensor_copy(dsb[half][:, :], dsb_inv[half][:, :])
                    for g in range(N_CORES):
                        a_raw = t1.tile([128, R], F32, tag="araw")
                        nc.sync.dma_start(
                            a_raw[0:64, :], a2a_out[2 * half][g, 0:64, :]
                        )
                        nc.sync.dma_start(
                            a_raw[64:128, :], a2a_out[2 * half + 1][g, 0:64, :]
                        )
                        bc = bc_pool.tile(
                            [128, QB], F32, tag=bc_tag, name=f"bc{half}_{g}"
                        )
                        nc.tensor.matmul(
                            bc[:, 0:R],
                            sel_sb[:, 2 * g + half, :],
                            dsb[half][:, :],
                            start=True, stop=True,
                        )
                        nc.vector.tensor_mul(
                            an_sb[:, 2 * g + half, :], a_raw[:], bc[:, 0:R]
                        )
                        yield

                att3 = emit_att(3)
                for unit in att3:
                    if unit == 0:
                        break
                # pair 0 stored: fire chunks 0,1 and prep half-0 an slices
                # (bc matmuls borrow the acc banks, idle with no proj left)
                emit_chunk(0)
                emit_chunk(1)
                emit_den(0)
                emit_den(1)
                an0 = an_half(0, psA, "acc")
                interleave(att3, an0, att_per_proj=2)
                drive(an0)
                emit_chunk(2)
                emit_chunk(3)
                emit_den(2)
                emit_den(3)

                # out projection in two nb-pair passes over acc+pv banks;
                # half-0 accumulation of pass 1 overlaps chunks 2/3
                def po_pass(nbs, emit_an1):
                    po = {}
                    for i, nb in enumerate(nbs):
                        po[nb] = [
                            (psA if i == 0 else psPV).tile(
                                [128, QB], F32,
                                tag=("acc" if i == 0 else "pv"),
                                name=f"po{q}_{nb}",
                            )
                            for q in range(2)
                        ]
                    for half in range(2):
                        for g in range(N_CORES):
                            gh = 2 * g + half
                            first = half == 0 and g == 0
                            last = half == 1 and g == N_CORES - 1
                            for nb in nbs:
                                for q in range(2):
                                    nc.tensor.matmul(
                                        po[nb][q][:],
                                        an_sb[:, gh, 128 * q : 128 * q + 128],
                                        wo_sb[:, gh, 512 * nb : 512 * nb + 512],
                                        start=first, stop=last,
                                    )
                        if half == 0 and emit_an1:
                            drive(an_half(1, psB, "sc"))
                    for nb in nbs:
                        for q in range(2):
                            osb = t1.tile([128, QB], F32, tag="osb")
                            nc.vector.tensor_copy(osb[:], po[nb][q][:])
                            nc.sync.dma_start(
                                out[128 * q : 128 * q + 128, 512 * nb : 512 * nb + 512],
                                osb[:],
                            )

                po_pass((0, 1), emit_an1=True)
                po_pass((2, 3), emit_an1=False)

    nc.compile()
    return nc


_NC_CACHE = None


def _get_nc():
    global _NC_CACHE
    if _NC_CACHE is None:
        _NC_CACHE = _build()
    return _NC_CACHE


def _to_ktile_layout(w):
    m = w.shape[1]
    return np.ascontiguousarray(w.reshape(KT_TILES, 128, m).transpose(1, 0, 2))


def _make_in_maps(x, cos, sin, wq, wk, wv, wo, q_norm_w, k_norm_w):
    x = np.asarray(x, dtype=np.float32)
    cos = np.asarray(cos, dtype=np.float32)
    sin = np.asarray(sin, dtype=np.float32)
    wq = np.asarray(wq, dtype=np.float32)
    wk = np.asarray(wk, dtype=np.float32)
    wv = np.asarray(wv, dtype=np.float32)
    wo = np.asarray(wo, dtype=np.float32)
    qw = np.asarray(q_norm_w, dtype=np.float32)
    kw = np.asarray(k_norm_w, dtype=np.float32)

    xT = np.ascontiguousarray(x[0].T).astype(BF16_NP)
    wo_b = _to_ktile_layout(wo).astype(BF16_NP)

    cosT = cos.T  # [64, SEQ]
    sinT = sin.T
    sgn = np.where(np.arange(64) < 32, -1.0, 1.0).astype(np.float32)
    wrot_q = qw[(np.arange(64) + 32) % 64]
    wrot_k = kw[(np.arange(64) + 32) % 64]
    cq1 = cosT * qw[:, None]
    sq1 = sinT * (sgn * wrot_q)[:, None]
    coswq = np.ascontiguousarray(np.vstack([cq1, cq1]))
    sinwq = np.ascontiguousarray(np.vstack([sq1, sq1]))
    coswk = np.ascontiguousarray(cosT * kw[:, None])
    sinwk = np.ascontiguousarray(sinT * (sgn * wrot_k)[:, None])

    ii, jj = np.meshgrid(np.arange(128), np.arange(128), indexing="ij")
    tri = np.where(ii <= jj, 0.0, NEG).astype(np.float32)
    onesblk = np.zeros((128, 128), np.float32)
    onesblk[0:64, 0:64] = 1.0
    onesblk[64:128, 64:128] = 1.0
    sel16 = np.zeros((2 * N_CORES, 2 * N_CORES, 128), np.float32)
    for g in range(N_CORES):
        for half in range(2):
            for m in range(128):
                sel16[8 * (m // 64) + g, 2 * g + half, m] = 1.0

    in_maps = []
    for c in range(N_CORES):
        wq_c = _to_ktile_layout(
            np.ascontiguousarray(wq[:, 256 * c : 256 * c + 256])
        ).astype(BF16_NP)
        wkv_c = _to_ktile_layout(
            np.ascontiguousarray(
                np.concatenate(
                    [wk[:, 64 * c : 64 * c + 64], wv[:, 64 * c : 64 * c + 64]],
                    axis=1,
                )
            )
        ).astype(BF16_NP)
        in_maps.append(
            {
                "xT": xT,
                "wq": wq_c,
                "wkv": wkv_c,
                "wo": wo_b,
                "coswq": coswq,
                "sinwq": sinwq,
                "coswk": coswk,
                "sinwk": sinwk,
                "tri": tri,
                "sel16": sel16,
                "onesblk": onesblk,
            }
        )
    return in_maps


def kernel(x, cos, sin, wq, wk, wv, wo, q_norm_w, k_norm_w):
    in_maps = _make_in_maps(x, cos, sin, wq, wk, wv, wo, q_norm_w, k_norm_w)
    nc = _get_nc()
    res = run_bass_kernel_spmd(nc, in_maps, core_ids=list(range(N_CORES)))
    rows = [res.results[c]["out"] for c in range(N_CORES)]
    full = np.concatenate(rows, axis=0)  # [SEQ, D_IN]
    return full.reshape(1, SEQ, D_IN).astype(np.float32)


# revision 24
# speedup vs baseline: 1.1186x; 1.0044x over previous
"""GQA FlashAttention (RMSNorm QK + RoPE, causal) on 8 TRN2 NeuronCores.

Sharding: tensor-parallel over heads (core c owns q-heads 4c..4c+3 and
kv-head c; the GQA group is fully local). Head-chunked AllToAlls re-shard
the attention output from head-parallel to row-parallel; each core then
computes its 256 output rows against the full Wo.

v5 vs v4:
- Row-packed scores: the K=64 scores matmuls for a pair of heads run
  concurrently in PE row-groups 0 and 64 (kt duplicated to both halves,
  qt stored pair-stacked), halving attention PE occupancy for the same
  ACT cost. This also removes the cross-partition RoPE writes.
- Two-pass projections (Q pass then KV pass over SBUF-resident x tiles)
  cut the live accumulator banks from 3 to 2, freeing PSUM for a second
  PV bank: 2 acc + 4 scores + 2 pv = 8 banks.
- Out projection runs inside the same pool scope: its half-0
  accumulation (chunks 0/1 of the AllToAll) executes in the freed acc
  banks while chunks 2/3 are still in flight.
"""

import sys

sys.path.insert(0, "/opt/trn_rl_repo")

import numpy as np
import ml_dtypes
import concourse.bass as bass  # noqa: F401
import concourse.tile as tile
from concourse import mybir, bacc
import concourse.bacc as bacc_mod
from concourse.bass_utils import run_bass_kernel_spmd
from concourse.hw_specs import get_activation_tables as _orig_get_tables
from concourse.masks import make_identity

N_CORES = 8
D_IN = 2048
SEQ = 2048
N_HEADS = 32
N_KV = 8
HD = 64
HPC = N_HEADS // N_CORES  # 4 q heads per core
EPS = 1e-6
NEG = -1.0e9

F32 = mybir.dt.float32
F32R = mybir.dt.float32r
BF16 = mybir.dt.bfloat16
BF16_NP = ml_dtypes.bfloat16

KT_TILES = D_IN // 128
QB = 512
NQB = SEQ // QB  # 4
ROWS_PER_CORE = SEQ // N_CORES  # 256
AF = mybir.ActivationFunctionType

_ONE_TABLE = "natural_log_exp_and_others"


def _pinned_tables(arch):
    tabs = _orig_get_tables(arch)
    return {n: (fs if n == _ONE_TABLE else set()) for n, fs in tabs.items()}


def _build():
    bacc_mod.get_activation_tables = _pinned_tables
    nc = bacc.Bacc(num_devices=N_CORES)

    xT = nc.dram_tensor("xT", [D_IN, SEQ], BF16, kind="ExternalInput")
    wq = nc.dram_tensor("wq", [128, KT_TILES, HPC * HD], BF16, kind="ExternalInput")
    wkv = nc.dram_tensor("wkv", [128, KT_TILES, 2 * HD], BF16, kind="ExternalInput")
    wo = nc.dram_tensor("wo", [128, KT_TILES, D_IN], BF16, kind="ExternalInput")
    coswq = nc.dram_tensor("coswq", [128, SEQ], F32, kind="ExternalInput")
    sinwq = nc.dram_tensor("sinwq", [128, SEQ], F32, kind="ExternalInput")
    coswk = nc.dram_tensor("coswk", [64, SEQ], F32, kind="ExternalInput")
    sinwk = nc.dram_tensor("sinwk", [64, SEQ], F32, kind="ExternalInput")
    tri = nc.dram_tensor("tri", [128, 128], F32, kind="ExternalInput")
    sel16 = nc.dram_tensor("sel16", [2 * N_CORES, 2 * N_CORES, 128], F32R, kind="ExternalInput")
    onesblk_in = nc.dram_tensor("onesblk", [128, 128], F32R, kind="ExternalInput")

    out = nc.dram_tensor("out", [ROWS_PER_CORE, D_IN], F32, kind="ExternalOutput")

    with tile.TileContext(nc) as tc:
        with (
            tc.tile_pool(name="persist", bufs=1) as pers,
            tc.tile_pool(name="dram", bufs=1, space="DRAM") as dram,
        ):
            # ---- persistent SBUF preloads (contiguous, host-transposed) ----
            # weights preload from the (otherwise idle) scalar queue so the
            # sync queue starts issuing x tiles immediately
            wq_sb = pers.tile([128, KT_TILES, HPC * HD], BF16)
            wkv_sb = pers.tile([128, KT_TILES, 2 * HD], BF16)
            nc.scalar.dma_start(wq_sb[:], wq[:])
            nc.scalar.dma_start(wkv_sb[:], wkv[:])

            cq_sb = pers.tile([128, SEQ], F32)
            sq_sb = pers.tile([128, SEQ], F32)
            ck_sb = pers.tile([64, SEQ], F32)
            sk_sb = pers.tile([64, SEQ], F32)
            nc.gpsimd.dma_start(cq_sb[:], coswq[:])
            nc.gpsimd.dma_start(sq_sb[:], sinwq[:])
            nc.gpsimd.dma_start(ck_sb[:], coswk[:])
            nc.gpsimd.dma_start(sk_sb[:], sinwk[:])
            tri_sb = pers.tile([128, 128], F32)
            nc.gpsimd.dma_start(tri_sb[:], tri[:])
            onesblk = pers.tile([128, 128], F32R)
            nc.gpsimd.dma_start(onesblk[:], onesblk_in[:])

            wo_sb = pers.tile([128, KT_TILES, D_IN], BF16)  # 8 MB
            sel_sb = pers.tile([2 * N_CORES, 2 * N_CORES, 128], F32R)

            ident = pers.tile([128, 128], F32)
            make_identity(nc, ident[:])
            eps_sb = pers.tile([128, 1], F32)
            nc.vector.memset(eps_sb[:], EPS)

            # pair-stacked q (pair p holds heads 2p/2p+1 in partition halves)
            qt = [pers.tile([128, 2, QB], BF16, name=f"qt{j}") for j in range(NQB)]
            # k duplicated into both partition halves for row-group packing
            kt = [pers.tile([128, QB], BF16, name=f"kt{j}") for j in range(NQB)]
            vaug = [pers.tile([128, 4, HD + 1], BF16, name=f"va{j}") for j in range(NQB)]

            a2a_in = [
                dram.tile([N_CORES, HD + 1, ROWS_PER_CORE], F32, name=f"a2ai{h}")
                for h in range(HPC)
            ]
            a2a_out = [
                dram.tile([N_CORES, HD + 1, ROWS_PER_CORE], F32, name=f"a2ao{h}")
                for h in range(HPC)
            ]
            cc_warm_in = dram.tile([N_CORES, 4], F32, name="ccwi")
            cc_warm_out = dram.tile([N_CORES, 4], F32, name="ccwo")

            # ============ fused projections + attention ====================
            with (
                tc.tile_pool(name="xt", bufs=8) as xp,
                tc.tile_pool(name="acc", bufs=2, space="PSUM") as psA,
                tc.tile_pool(name="sc", bufs=2, space="PSUM") as psB,
                tc.tile_pool(name="pv", bufs=2, space="PSUM") as psPV,
                tc.tile_pool(name="work", bufs=2) as t1,
                tc.tile_pool(name="ptp", bufs=3) as ptp,
            ):

                def norm_rope(j, raw_psum, idx):
                    """Evict + rmsnorm + rope one accumulator. idx 0/1 = q
                    pairs, idx 2 = kv. Generator (yields mid-chain)."""
                    sl = slice(QB * j, QB * j + QB)
                    is_kv = idx == 2
                    rows = slice(0, 64) if is_kv else slice(0, 128)
                    rawsb = t1.tile([128, QB], F32, tag="rawsb")
                    nc.vector.tensor_copy(rawsb[:], raw_psum[:])
                    sq = t1.tile([128, QB], F32R, tag="sq")
                    nc.vector.tensor_mul(sq[:], rawsb[:], rawsb[:])
                    psn = psB.tile([128, 2, QB], F32, tag="sc", name=f"psn{idx}_{j}")
                    nc.tensor.matmul(psn[:, 0, :], onesblk[:], sq[:], start=True, stop=True)
                    lnv = t1.tile([128, QB], F32, tag="lnv", bufs=1)
                    nc.scalar.activation(
                        out=lnv[rows, :], in_=psn[rows, 0, :],
                        func=AF.Ln, bias=eps_sb[rows, :], scale=1.0 / HD,
                    )
                    rcp = t1.tile([128, QB], F32, tag="rcp", bufs=1)
                    nc.scalar.activation(
                        out=rcp[rows, :], in_=lnv[rows, :], func=AF.Exp, scale=-0.5,
                    )
                    yield
                    tn = t1.tile([128, QB], F32, tag="tn")
                    nc.vector.tensor_mul(tn[rows, :], rawsb[rows, :], rcp[rows, :])
                    rot = t1.tile([128, QB], F32, tag="rot")
                    nh = 1 if is_kv else 2
                    for b in range(nh):
                        o = 64 * b
                        nc.vector.tensor_copy(rot[o : o + 32, :], tn[o + 32 : o + 64, :])
                        nc.vector.tensor_copy(rot[o + 32 : o + 64, :], tn[o : o + 32, :])
                    if is_kv:
                        tcs = t1.tile([64, QB], F32, tag="tcs", bufs=1)
                        nc.vector.tensor_mul(tcs[:], tn[0:64, :], ck_sb[:, sl])
                        nc.vector.tensor_mul(rot[0:64, :], rot[0:64, :], sk_sb[:, sl])
                        nc.vector.tensor_add(kt[j][0:64, :], tcs[:], rot[0:64, :])
                        nc.vector.tensor_copy(kt[j][64:128, :], kt[j][0:64, :])
                        vt = t1.tile([64, QB], F32, tag="vt", bufs=1)
                        nc.vector.tensor_copy(vt[:], rawsb[64:128, :])
                        for d in range(4):
                            psv = psB.tile([128, 2, QB], F32, tag="sc", name=f"psv{j}_{d}")
                            nc.tensor.transpose(
                                psv[:, 0, 0:64],
                                vt[:, 128 * d : 128 * d + 128],
                                ident[0:64, 0:64],
                            )
                            nc.vector.tensor_copy(vaug[j][:, d, 0:HD], psv[:, 0, 0:64])
                            nc.vector.memset(vaug[j][:, d, HD : HD + 1], 1.0)
                            if d == 1:
                                yield
                    else:
                        tc2 = t1.tile([128, QB], F32, tag="tc2")
                        nc.vector.tensor_mul(tc2[:], tn[:], cq_sb[:, sl])
                        nc.vector.tensor_mul(rot[:], rot[:], sq_sb[:, sl])
                        nc.vector.tensor_add(qt[j][:, idx, :], tc2[:], rot[:])
                    yield

                def emit_proj(j):
                    """Two-pass projection: Q (2 banks) then KV (1 bank).
                    The KV pass reloads x from HBM (bandwidth is cheap,
                    SBUF slots and slot-WAR stalls are not)."""
                    sl = slice(QB * j, QB * j + QB)
                    accq = [
                        psA.tile([128, QB], F32, tag="acc", name=f"accq{i}_{j}")
                        for i in range(2)
                    ]
                    for k in range(KT_TILES):
                        xt = xp.tile([128, QB], BF16, tag="xt", name=f"xt{j}_{k}")
                        nc.sync.dma_start(xt[:], xT[128 * k : 128 * k + 128, sl])
                        st = k == 0
                        sp = k == KT_TILES - 1
                        nc.tensor.matmul(accq[0][:], wq_sb[:, k, 0:128], xt[:], start=st, stop=sp)
                        nc.tensor.matmul(accq[1][:], wq_sb[:, k, 128:256], xt[:], start=st, stop=sp)
                        if k % 2 == 1:
                            yield
                    yield from norm_rope(j, accq[0], 0)
                    yield from norm_rope(j, accq[1], 1)
                    acckv = psA.tile([128, QB], F32, tag="acc", name=f"acckv_{j}")
                    for k in range(KT_TILES):
                        xt2 = xp.tile([128, QB], BF16, tag="xt2", name=f"xu{j}_{k}", bufs=6)
                        nc.sync.dma_start(xt2[:], xT[128 * k : 128 * k + 128, sl])
                        st = k == 0
                        sp = k == KT_TILES - 1
                        nc.tensor.matmul(acckv[:], wkv_sb[:, k, :], xt2[:], start=st, stop=sp)
                        if k % 4 == 3:
                            yield
                    yield from norm_rope(j, acckv, 2)

                def emit_att(j):
                    """Attention for block j, head-pair packed: scores for
                    heads 2p/2p+1 run concurrently in PE row groups 0/64.
                    Yields after each unit; yields (pair index) after a
                    pair's output is fully stored."""
                    for p in range(2):
                        pvs = [
                            psPV.tile([128, QB], F32, tag="pv", name=f"pv{j}_{p}_{u}")
                            for u in range(2)
                        ]
                        ntile = 4 * j + 4
                        for t in range(ntile):
                            jj, d = t // 4, t % 4
                            diag = jj == j
                            n0 = 128 * d if diag else 0
                            w = QB - n0
                            sc = psB.tile([128, 2, QB], F32, tag="sc", name=f"sc{j}_{p}_{t}")
                            for u in range(2):
                                nc.tensor.matmul(
                                    sc[:, u, 0:w],
                                    kt[jj][64 * u : 64 * u + 64, 128 * d : 128 * d + 128],
                                    qt[j][64 * u : 64 * u + 64, p, n0:QB],
                                    start=True, stop=True,
                                )
                            if diag:
                                for u in range(2):
                                    nc.vector.tensor_add(
                                        sc[:, u, 0:128], sc[:, u, 0:128], tri_sb[:]
                                    )
                            pt = ptp.tile([128, 2, QB], BF16, tag="pt")
                            nc.scalar.activation(
                                out=pt[:, :, 0:w], in_=sc[:, :, 0:w],
                                func=AF.Exp, scale=0.125,
                            )
                            for u in range(2):
                                nc.tensor.matmul(
                                    pvs[u][0:65, n0:QB],
                                    vaug[jj][:, d, :],
                                    pt[:, u, 0:w],
                                    start=(t == 0), stop=(t == ntile - 1),
                                )
                            if t % 2 == 1 or diag:
                                yield
                        for u in range(2):
                            h = 2 * p + u
                            att = t1.tile([65, QB], F32, tag="att")
                            nc.vector.tensor_copy(att[:], pvs[u][0:65, :])
                            for s in range(2):
                                shard = 2 * j + s
                                cs = slice(ROWS_PER_CORE * s, ROWS_PER_CORE * (s + 1))
                                nc.gpsimd.dma_start(
                                    a2a_in[h][shard, 0:64, :], att[0:64, cs]
                                )
                                nc.gpsimd.dma_start(
                                    a2a_in[h][shard, 64, :], att[64:65, cs]
                                )
                        yield p

                def drive(gen):
                    for _ in gen:
                        pass

                def interleave(att_gen, proj_gen, att_per_proj=1):
                    att_done = proj_done = False
                    while not (att_done and proj_done):
                        for _ in range(att_per_proj):
                            if not att_done:
                                att_done = next(att_gen, "END") == "END"
                        if not proj_done:
                            proj_done = next(proj_gen, "END") == "END"

                # warmup collective: the first collective after load pays
                # ~25-30us of one-time ncfw setup; prepay it during compute
                nc.gpsimd.collective_compute(
                    "AllToAll",
                    mybir.AluOpType.bypass,
                    replica_groups=[list(range(N_CORES))],
                    ins=[cc_warm_in[:].opt()],
                    outs=[cc_warm_out[:].opt()],
                )

                drive(emit_proj(0))
                nc.scalar.dma_start(sel_sb[:], sel16[:])
                interleave(emit_att(0), emit_proj(1), att_per_proj=1)
                # wo preload issues from the gpsimd queue after att(0)'s
                # stores - late enough not to steal HBM bandwidth from the
                # first blocks, early enough to land before out-projection
                nc.gpsimd.dma_start(wo_sb[:], wo[:])
                interleave(emit_att(1), emit_proj(2), att_per_proj=1)
                interleave(emit_att(2), emit_proj(3), att_per_proj=1)

                # ---- last block's attention + pipelined reshard/out-proj ----
                R = ROWS_PER_CORE
                dsb_raw = [
                    t1.tile([2 * N_CORES, R], F32, tag=f"denraw{i}", name=f"denraw{i}", bufs=1)
                    for i in range(2)
                ]
                dsb_inv = [
                    t1.tile([2 * N_CORES, R], F32, tag=f"deninv{i}", name=f"deninv{i}", bufs=1)
                    for i in range(2)
                ]
                dsb = [
                    t1.tile([2 * N_CORES, R], F32R, tag=f"den{i}", name=f"den{i}", bufs=1)
                    for i in range(2)
                ]
                an_sb = pers.tile([128, 2 * N_CORES, R], BF16)

                def emit_chunk(h):
                    nc.gpsimd.collective_compute(
                        "AllToAll",
                        mybir.AluOpType.bypass,
                        replica_groups=[list(range(N_CORES))],
                        ins=[a2a_in[h][:].opt()],
                        outs=[a2a_out[h][:].opt()],
                    )

                def emit_den(h):
                    hf, rs = h // 2, slice(8 * (h % 2), 8 * (h % 2) + 8)
                    nc.sync.dma_start(dsb_raw[hf][rs, :], a2a_out[h][:, 64, :])

                def an_half(half, bc_pool, bc_tag):
                    """Generator: normalize the attnT slices for chunk pair
                    `half` (one unit per source core)."""
                    nc.vector.reciprocal_approx_fast(
                        out=dsb_inv[half][:, :], in_=dsb_raw[half][:, :]
                    )
                    nc.vector.tensor_copy(dsb[half][:, :], dsb_inv[half][:, :])
                    for g in range(N_CORES):
                        a_raw = t1.tile([128, R], F32, tag="araw")
                        nc.sync.dma_start(
                            a_raw[0:64, :], a2a_out[2 * half][g, 0:64, :]
                        )
                        nc.sync.dma_start(
                            a_raw[64:128, :], a2a_out[2 * half + 1][g, 0:64, :]
                        )
                        bc = bc_pool.tile(
                            [128, QB], F32, tag=bc_tag, name=f"bc{half}_{g}"
                        )
                        nc.tensor.matmul(
                            bc[:, 0:R],
                            sel_sb[:, 2 * g + half, :],
                            dsb[half][:, :],
                            start=True, stop=True,
                        )
                        nc.vector.tensor_mul(
                            an_sb[:, 2 * g + half, :], a_raw[:], bc[:, 0:R]
                        )
                        yield

                att3 = emit_att(3)
                for unit in att3:
                    if unit == 0:
                        break
                # pair 0 stored: fire chunks 0,1 and prep half-0 an slices
                # (bc matmuls borrow the acc banks, idle with no proj left)
                emit_chunk(0)
                emit_chunk(1)
                emit_den(0)
                emit_den(1)
                an0 = an_half(0, psA, "acc")
                interleave(att3, an0, att_per_proj=2)
                drive(an0)
                emit_chunk(2)
                emit_chunk(3)
                emit_den(2)
                emit_den(3)

                # out projection: nb 0-2 get six accumulators (acc, pv, and
                # both halves of one sc slot) so all their half-0 work runs
                # under the chunk-2/3 transfers; the an-half-1 bc matmuls
                # use the second sc slot; nb3 runs last in the acc slots.
                poA = [psA.tile([128, QB], F32, tag="acc", name=f"poA{q}") for q in range(2)]
                poB = [psPV.tile([128, QB], F32, tag="pv", name=f"poB{q}") for q in range(2)]
                poCt = psB.tile([128, 2, QB], F32, tag="sc", name="poC")
                po_aps = {
                    0: [poA[0][:], poA[1][:]],
                    1: [poB[0][:], poB[1][:]],
                    2: [poCt[:, 0, :], poCt[:, 1, :]],
                }

                def po_mm(nb, q, gh, first, last):
                    nc.tensor.matmul(
                        po_aps[nb][q],
                        an_sb[:, gh, 128 * q : 128 * q + 128],
                        wo_sb[:, gh, 512 * nb : 512 * nb + 512],
                        start=first, stop=last,
                    )

                for g in range(N_CORES):
                    for nb in range(3):
                        for q in range(2):
                            po_mm(nb, q, 2 * g, g == 0, False)
                an1 = an_half(1, psB, "sc")
                for g in range(N_CORES):
                    next(an1, None)
                    for nb in range(3):
                        for q in range(2):
                            po_mm(nb, q, 2 * g + 1, False, g == N_CORES - 1)
                drive(an1)

                def po_evict(nb):
                    for q in range(2):
                        osb = t1.tile([128, QB], F32, tag="osb")
                        nc.vector.tensor_copy(osb[:], po_aps[nb][q])
                        nc.sync.dma_start(
                            out[128 * q : 128 * q + 128, 512 * nb : 512 * nb + 512],
                            osb[:],
                        )

                po_evict(0)
                po_last = [psA.tile([128, QB], F32, tag="acc", name=f"poD{q}") for q in range(2)]
                po_aps[3] = [po_last[0][:], po_last[1][:]]
                for half in range(2):
                    for g in range(N_CORES):
                        gh = 2 * g + half
                        for q in range(2):
                            po_mm(3, q, gh, half == 0 and g == 0,
                                  half == 1 and g == N_CORES - 1)
                po_evict(1)
                po_evict(2)
                po_evict(3)

    nc.compile()
    return nc


_NC_CACHE = None


def _get_nc():
    global _NC_CACHE
    if _NC_CACHE is None:
        _NC_CACHE = _build()
    return _NC_CACHE


def _to_ktile_layout(w):
    m = w.shape[1]
    return np.ascontiguousarray(w.reshape(KT_TILES, 128, m).transpose(1, 0, 2))


def _make_in_maps(x, cos, sin, wq, wk, wv, wo, q_norm_w, k_norm_w):
    x = np.asarray(x, dtype=np.float32)
    cos = np.asarray(cos, dtype=np.float32)
    sin = np.asarray(sin, dtype=np.float32)
    wq = np.asarray(wq, dtype=np.float32)
    wk = np.asarray(wk, dtype=np.float32)
    wv = np.asarray(wv, dtype=np.float32)
    wo = np.asarray(wo, dtype=np.float32)
    qw = np.asarray(q_norm_w, dtype=np.float32)
    kw = np.asarray(k_norm_w, dtype=np.float32)

    xT = np.ascontiguousarray(x[0].T).astype(BF16_NP)
    wo_b = _to_ktile_layout(wo).astype(BF16_NP)

    cosT = cos.T  # [64, SEQ]
    sinT = sin.T
    sgn = np.where(np.arange(64) < 32, -1.0, 1.0).astype(np.float32)
    wrot_q = qw[(np.arange(64) + 32) % 64]
    wrot_k = kw[(np.arange(64) + 32) % 64]
    cq1 = cosT * qw[:, None]
    sq1 = sinT * (sgn * wrot_q)[:, None]
    coswq = np.ascontiguousarray(np.vstack([cq1, cq1]))
    sinwq = np.ascontiguousarray(np.vstack([sq1, sq1]))
    coswk = np.ascontiguousarray(cosT * kw[:, None])
    sinwk = np.ascontiguousarray(sinT * (sgn * wrot_k)[:, None])

    ii, jj = np.meshgrid(np.arange(128), np.arange(128), indexing="ij")
    tri = np.where(ii <= jj, 0.0, NEG).astype(np.float32)
    onesblk = np.zeros((128, 128), np.float32)
    onesblk[0:64, 0:64] = 1.0
    onesblk[64:128, 64:128] = 1.0
    sel16 = np.zeros((2 * N_CORES, 2 * N_CORES, 128), np.float32)
    for g in range(N_CORES):
        for half in range(2):
            for m in range(128):
                sel16[8 * (m // 64) + g, 2 * g + half, m] = 1.0

    in_maps = []
    for c in range(N_CORES):
        wq_c = _to_ktile_layout(
            np.ascontiguousarray(wq[:, 256 * c : 256 * c + 256])
        ).astype(BF16_NP)
        wkv_c = _to_ktile_layout(
            np.ascontiguousarray(
                np.concatenate(
                    [wk[:, 64 * c : 64 * c + 64], wv[:, 64 * c : 64 * c + 64]],
                    axis=1,
                )
            )
        ).astype(BF16_NP)
        in_maps.append(
            {
                "xT": xT,
                "wq": wq_c,
                "wkv": wkv_c,
                "wo": wo_b,
                "coswq": coswq,
                "sinwq": sinwq,
                "coswk": coswk,
                "sinwk": sinwk,
                "tri": tri,
                "sel16": sel16,
                "onesblk": onesblk,
            }
        )
    return in_maps


def kernel(x, cos, sin, wq, wk, wv, wo, q_norm_w, k_norm_w):
    in_maps = _make_in_maps(x, cos, sin, wq, wk, wv, wo, q_norm_w, k_norm_w)
    nc = _get_nc()
    res = run_bass_kernel_spmd(nc, in_maps, core_ids=list(range(N_CORES)))
    rows = [res.results[c]["out"] for c in range(N_CORES)]
    full = np.concatenate(rows, axis=0)  # [SEQ, D_IN]
    return full.reshape(1, SEQ, D_IN).astype(np.float32)


# revision 26
# speedup vs baseline: 1.1411x; 1.0201x over previous
"""GQA FlashAttention (RMSNorm QK + RoPE, causal) on 8 TRN2 NeuronCores.

Sharding: tensor-parallel over heads (core c owns q-heads 4c..4c+3 and
kv-head c; the GQA group is fully local). Head-chunked AllToAlls re-shard
the attention output from head-parallel to row-parallel; each core then
computes its 256 output rows against the full Wo.

v5 vs v4:
- Row-packed scores: the K=64 scores matmuls for a pair of heads run
  concurrently in PE row-groups 0 and 64 (kt duplicated to both halves,
  qt stored pair-stacked), halving attention PE occupancy for the same
  ACT cost. This also removes the cross-partition RoPE writes.
- Two-pass projections (Q pass then KV pass over SBUF-resident x tiles)
  cut the live accumulator banks from 3 to 2, freeing PSUM for a second
  PV bank: 2 acc + 4 scores + 2 pv = 8 banks.
- Out projection runs inside the same pool scope: its half-0
  accumulation (chunks 0/1 of the AllToAll) executes in the freed acc
  banks while chunks 2/3 are still in flight.
"""

import sys

sys.path.insert(0, "/opt/trn_rl_repo")

import numpy as np
import ml_dtypes
import concourse.bass as bass  # noqa: F401
import concourse.tile as tile
from concourse import mybir, bacc
import concourse.bacc as bacc_mod
from concourse.bass_utils import run_bass_kernel_spmd
from concourse.hw_specs import get_activation_tables as _orig_get_tables
from concourse.masks import make_identity

N_CORES = 8
D_IN = 2048
SEQ = 2048
N_HEADS = 32
N_KV = 8
HD = 64
HPC = N_HEADS // N_CORES  # 4 q heads per core
EPS = 1e-6
NEG = -1.0e9

F32 = mybir.dt.float32
F32R = mybir.dt.float32r
BF16 = mybir.dt.bfloat16
BF16_NP = ml_dtypes.bfloat16

KT_TILES = D_IN // 128
QB = 512
NQB = SEQ // QB  # 4
ROWS_PER_CORE = SEQ // N_CORES  # 256
AF = mybir.ActivationFunctionType

_ONE_TABLE = "natural_log_exp_and_others"


def _pinned_tables(arch):
    tabs = _orig_get_tables(arch)
    return {n: (fs if n == _ONE_TABLE else set()) for n, fs in tabs.items()}


def _build():
    bacc_mod.get_activation_tables = _pinned_tables
    nc = bacc.Bacc(num_devices=N_CORES)

    xT = nc.dram_tensor("xT", [D_IN, SEQ], BF16, kind="ExternalInput")
    wq = nc.dram_tensor("wq", [128, KT_TILES, HPC * HD], BF16, kind="ExternalInput")
    wkv = nc.dram_tensor("wkv", [128, KT_TILES, 2 * HD], BF16, kind="ExternalInput")
    wo = nc.dram_tensor("wo", [128, KT_TILES, D_IN], BF16, kind="ExternalInput")
    coswq = nc.dram_tensor("coswq", [128, SEQ], BF16, kind="ExternalInput")
    sinwq = nc.dram_tensor("sinwq", [128, SEQ], BF16, kind="ExternalInput")
    coswk = nc.dram_tensor("coswk", [64, SEQ], BF16, kind="ExternalInput")
    sinwk = nc.dram_tensor("sinwk", [64, SEQ], BF16, kind="ExternalInput")
    tri = nc.dram_tensor("tri", [128, 128], F32, kind="ExternalInput")
    sel16 = nc.dram_tensor("sel16", [2 * N_CORES, 2 * N_CORES, 128], F32R, kind="ExternalInput")
    onesblk_in = nc.dram_tensor("onesblk", [128, 128], F32R, kind="ExternalInput")

    out = nc.dram_tensor("out", [ROWS_PER_CORE, D_IN], F32, kind="ExternalOutput")

    with tile.TileContext(nc) as tc:
        with (
            tc.tile_pool(name="persist", bufs=1) as pers,
            tc.tile_pool(name="dram", bufs=1, space="DRAM") as dram,
        ):
            # ---- persistent SBUF preloads (contiguous, host-transposed) ----
            # weights preload from the (otherwise idle) scalar queue so the
            # sync queue starts issuing x tiles immediately
            wq_sb = pers.tile([128, KT_TILES, HPC * HD], BF16)
            wkv_sb = pers.tile([128, KT_TILES, 2 * HD], BF16)
            nc.scalar.dma_start(wq_sb[:], wq[:])
            nc.scalar.dma_start(wkv_sb[:], wkv[:])

            cq_sb = pers.tile([128, SEQ], BF16)
            sq_sb = pers.tile([128, SEQ], BF16)
            ck_sb = pers.tile([64, SEQ], BF16)
            sk_sb = pers.tile([64, SEQ], BF16)
            tri_sb = pers.tile([128, 128], F32)
            onesblk = pers.tile([128, 128], F32R)

            def load_rope_tables():
                # issued on the sync queue AFTER proj(0)'s pass-1 x loads:
                # in-order issue keeps the first matmuls' data ahead of
                # these on the shared HBM pipe
                nc.sync.dma_start(onesblk[:], onesblk_in[:])
                nc.sync.dma_start(cq_sb[:], coswq[:])
                nc.sync.dma_start(sq_sb[:], sinwq[:])
                nc.sync.dma_start(ck_sb[:], coswk[:])
                nc.sync.dma_start(sk_sb[:], sinwk[:])
                nc.sync.dma_start(tri_sb[:], tri[:])

            wo_sb = pers.tile([128, KT_TILES, D_IN], BF16)  # 8 MB
            sel_sb = pers.tile([2 * N_CORES, 2 * N_CORES, 128], F32R)

            ident = pers.tile([128, 128], F32)
            make_identity(nc, ident[:])
            eps_sb = pers.tile([128, 1], F32)
            nc.vector.memset(eps_sb[:], EPS)

            # pair-stacked q (pair p holds heads 2p/2p+1 in partition halves)
            qt = [pers.tile([128, 2, QB], BF16, name=f"qt{j}") for j in range(NQB)]
            # k duplicated into both partition halves for row-group packing
            kt = [pers.tile([128, QB], BF16, name=f"kt{j}") for j in range(NQB)]
            vaug = [pers.tile([128, 4, HD + 1], BF16, name=f"va{j}") for j in range(NQB)]

            a2a_in = [
                dram.tile([N_CORES, HD + 1, ROWS_PER_CORE], BF16, name=f"a2ai{h}")
                for h in range(HPC)
            ]
            a2a_out = [
                dram.tile([N_CORES, HD + 1, ROWS_PER_CORE], BF16, name=f"a2ao{h}")
                for h in range(HPC)
            ]
            cc_warm_in = dram.tile([N_CORES, 4], F32, name="ccwi")
            cc_warm_out = dram.tile([N_CORES, 4], F32, name="ccwo")

            # ============ fused projections + attention ====================
            with (
                tc.tile_pool(name="xt", bufs=8) as xp,
                tc.tile_pool(name="acc", bufs=2, space="PSUM") as psA,
                tc.tile_pool(name="sc", bufs=2, space="PSUM") as psB,
                tc.tile_pool(name="pv", bufs=2, space="PSUM") as psPV,
                tc.tile_pool(name="work", bufs=2) as t1,
                tc.tile_pool(name="ptp", bufs=3) as ptp,
            ):

                def norm_rope(j, raw_psum, idx):
                    """Evict + rmsnorm + rope one accumulator. idx 0/1 = q
                    pairs, idx 2 = kv. Generator (yields mid-chain)."""
                    sl = slice(QB * j, QB * j + QB)
                    is_kv = idx == 2
                    rows = slice(0, 64) if is_kv else slice(0, 128)
                    rawsb = t1.tile([128, QB], F32, tag="rawsb")
                    nc.vector.tensor_copy(rawsb[:], raw_psum[:])
                    sq = t1.tile([128, QB], F32R, tag="sq")
                    nc.vector.tensor_mul(sq[:], rawsb[:], rawsb[:])
                    psn = psB.tile([128, 2, QB], F32, tag="sc", name=f"psn{idx}_{j}")
                    nc.tensor.matmul(psn[:, 0, :], onesblk[:], sq[:], start=True, stop=True)
                    lnv = t1.tile([128, QB], F32, tag="lnv", bufs=1)
                    nc.scalar.activation(
                        out=lnv[rows, :], in_=psn[rows, 0, :],
                        func=AF.Ln, bias=eps_sb[rows, :], scale=1.0 / HD,
                    )
                    rcp = t1.tile([128, QB], F32, tag="rcp", bufs=1)
                    nc.scalar.activation(
                        out=rcp[rows, :], in_=lnv[rows, :], func=AF.Exp, scale=-0.5,
                    )
                    yield
                    tn = t1.tile([128, QB], BF16, tag="tn")
                    nc.vector.tensor_mul(tn[rows, :], rawsb[rows, :], rcp[rows, :])
                    rot = t1.tile([128, QB], BF16, tag="rot")
                    nh = 1 if is_kv else 2
                    for b in range(nh):
                        o = 64 * b
                        nc.vector.tensor_copy(rot[o : o + 32, :], tn[o + 32 : o + 64, :])
                        nc.vector.tensor_copy(rot[o + 32 : o + 64, :], tn[o : o + 32, :])
                    if is_kv:
                        tcs = t1.tile([64, QB], BF16, tag="tcs", bufs=1)
                        nc.vector.tensor_mul(tcs[:], tn[0:64, :], ck_sb[:, sl])
                        nc.vector.tensor_mul(rot[0:64, :], rot[0:64, :], sk_sb[:, sl])
                        nc.vector.tensor_add(kt[j][0:64, :], tcs[:], rot[0:64, :])
                        nc.vector.tensor_copy(kt[j][64:128, :], kt[j][0:64, :])
                        vt = t1.tile([64, QB], F32, tag="vt", bufs=1)
                        nc.vector.tensor_copy(vt[:], rawsb[64:128, :])
                        for d in range(4):
                            psv = psB.tile([128, 2, QB], F32, tag="sc", name=f"psv{j}_{d}")
                            nc.tensor.transpose(
                                psv[:, 0, 0:64],
                                vt[:, 128 * d : 128 * d + 128],
                                ident[0:64, 0:64],
                            )
                            nc.vector.tensor_copy(vaug[j][:, d, 0:HD], psv[:, 0, 0:64])
                            nc.vector.memset(vaug[j][:, d, HD : HD + 1], 1.0)
                            if d == 1:
                                yield
                    else:
                        tc2 = t1.tile([128, QB], BF16, tag="tc2")
                        nc.vector.tensor_mul(tc2[:], tn[:], cq_sb[:, sl])
                        nc.vector.tensor_mul(rot[:], rot[:], sq_sb[:, sl])
                        nc.vector.tensor_add(qt[j][:, idx, :], tc2[:], rot[:])
                    yield

                def emit_proj(j):
                    """Two-pass projection: Q (2 banks) then KV (1 bank).
                    The KV pass reloads x from HBM (bandwidth is cheap,
                    SBUF slots and slot-WAR stalls are not)."""
                    sl = slice(QB * j, QB * j + QB)
                    accq = [
                        psA.tile([128, QB], F32, tag="acc", name=f"accq{i}_{j}")
                        for i in range(2)
                    ]
                    for k in range(KT_TILES):
                        xt = xp.tile([128, QB], BF16, tag="xt", name=f"xt{j}_{k}")
                        nc.sync.dma_start(xt[:], xT[128 * k : 128 * k + 128, sl])
                        st = k == 0
                        sp = k == KT_TILES - 1
                        nc.tensor.matmul(accq[0][:], wq_sb[:, k, 0:128], xt[:], start=st, stop=sp)
                        nc.tensor.matmul(accq[1][:], wq_sb[:, k, 128:256], xt[:], start=st, stop=sp)
                        if k % 2 == 1:
                            yield
                    if j == 0:
                        load_rope_tables()
                    yield from norm_rope(j, accq[0], 0)
                    yield from norm_rope(j, accq[1], 1)
                    acckv = psA.tile([128, QB], F32, tag="acc", name=f"acckv_{j}")
                    for k in range(KT_TILES):
                        xt2 = xp.tile([128, QB], BF16, tag="xt2", name=f"xu{j}_{k}", bufs=6)
                        nc.sync.dma_start(xt2[:], xT[128 * k : 128 * k + 128, sl])
                        st = k == 0
                        sp = k == KT_TILES - 1
                        nc.tensor.matmul(acckv[:], wkv_sb[:, k, :], xt2[:], start=st, stop=sp)
                        if k % 4 == 3:
                            yield
                    yield from norm_rope(j, acckv, 2)

                def emit_att(j):
                    """Attention for block j, head-pair packed: scores for
                    heads 2p/2p+1 run concurrently in PE row groups 0/64.
                    Yields after each unit; yields (pair index) after a
                    pair's output is fully stored."""
                    for p in range(2):
                        pvs = [
                            psPV.tile([128, QB], F32, tag="pv", name=f"pv{j}_{p}_{u}")
                            for u in range(2)
                        ]
                        ntile = 4 * j + 4
                        for t in range(ntile):
                            jj, d = t // 4, t % 4
                            diag = jj == j
                            n0 = 128 * d if diag else 0
                            w = QB - n0
                            sc = psB.tile([128, 2, QB], F32, tag="sc", name=f"sc{j}_{p}_{t}")
                            for u in range(2):
                                nc.tensor.matmul(
                                    sc[:, u, 0:w],
                                    kt[jj][64 * u : 64 * u + 64, 128 * d : 128 * d + 128],
                                    qt[j][64 * u : 64 * u + 64, p, n0:QB],
                                    start=True, stop=True,
                                )
                            if diag:
                                for u in range(2):
                                    nc.vector.tensor_add(
                                        sc[:, u, 0:128], sc[:, u, 0:128], tri_sb[:]
                                    )
                            pt = ptp.tile([128, 2, QB], BF16, tag="pt")
                            nc.scalar.activation(
                                out=pt[:, :, 0:w], in_=sc[:, :, 0:w],
                                func=AF.Exp, scale=0.125,
                            )
                            for u in range(2):
                                nc.tensor.matmul(
                                    pvs[u][0:65, n0:QB],
                                    vaug[jj][:, d, :],
                                    pt[:, u, 0:w],
                                    start=(t == 0), stop=(t == ntile - 1),
                                )
                            if t % 2 == 1 or diag:
                                yield
                        for u in range(2):
                            h = 2 * p + u
                            att = t1.tile([65, QB], BF16, tag="att")
                            nc.vector.tensor_copy(att[:], pvs[u][0:65, :])
                            for s in range(2):
                                shard = 2 * j + s
                                cs = slice(ROWS_PER_CORE * s, ROWS_PER_CORE * (s + 1))
                                nc.gpsimd.dma_start(
                                    a2a_in[h][shard, 0:64, :], att[0:64, cs]
                                )
                                nc.gpsimd.dma_start(
                                    a2a_in[h][shard, 64, :], att[64:65, cs]
                                )
                        yield p

                def drive(gen):
                    for _ in gen:
                        pass

                def interleave(att_gen, proj_gen, att_per_proj=1):
                    att_done = proj_done = False
                    while not (att_done and proj_done):
                        for _ in range(att_per_proj):
                            if not att_done:
                                att_done = next(att_gen, "END") == "END"
                        if not proj_done:
                            proj_done = next(proj_gen, "END") == "END"

                # warmup collective: the first collective after load pays
                # ~25-30us of one-time ncfw setup; prepay it during compute
                nc.gpsimd.collective_compute(
                    "AllToAll",
                    mybir.AluOpType.bypass,
                    replica_groups=[list(range(N_CORES))],
                    ins=[cc_warm_in[:].opt()],
                    outs=[cc_warm_out[:].opt()],
                )

                drive(emit_proj(0))
                nc.scalar.dma_start(sel_sb[:], sel16[:])
                interleave(emit_att(0), emit_proj(1), att_per_proj=1)
                # wo preload issues from the gpsimd queue after att(0)'s
                # stores - late enough not to steal HBM bandwidth from the
                # first blocks, early enough to land before out-projection
                nc.gpsimd.dma_start(wo_sb[:], wo[:])
                interleave(emit_att(1), emit_proj(2), att_per_proj=1)
                interleave(emit_att(2), emit_proj(3), att_per_proj=1)

                # ---- last block's attention + pipelined reshard/out-proj ----
                R = ROWS_PER_CORE
                dsb_raw = [
                    t1.tile([2 * N_CORES, R], BF16, tag=f"denraw{i}", name=f"denraw{i}", bufs=1)
                    for i in range(2)
                ]
                dsb_inv = [
                    t1.tile([2 * N_CORES, R], F32, tag=f"deninv{i}", name=f"deninv{i}", bufs=1)
                    for i in range(2)
                ]
                dsb = [
                    t1.tile([2 * N_CORES, R], F32R, tag=f"den{i}", name=f"den{i}", bufs=1)
                    for i in range(2)
                ]
                an_sb = pers.tile([128, 2 * N_CORES, R], BF16)

                def emit_chunk(h):
                    nc.gpsimd.collective_compute(
                        "AllToAll",
                        mybir.AluOpType.bypass,
                        replica_groups=[list(range(N_CORES))],
                        ins=[a2a_in[h][:].opt()],
                        outs=[a2a_out[h][:].opt()],
                    )

                def emit_den(h):
                    hf, rs = h // 2, slice(8 * (h % 2), 8 * (h % 2) + 8)
                    nc.sync.dma_start(dsb_raw[hf][rs, :], a2a_out[h][:, 64, :])

                def an_half(half, bc_pool, bc_tag):
                    """Generator: normalize the attnT slices for chunk pair
                    `half` (one unit per source core)."""
                    dcast = t1.tile([2 * N_CORES, R], F32, tag=f"dcast{half}",
                                    name=f"dcast{half}", bufs=1)
                    nc.vector.tensor_copy(dcast[:], dsb_raw[half][:, :])
                    nc.vector.reciprocal_approx_fast(
                        out=dsb_inv[half][:, :], in_=dcast[:]
                    )
                    nc.vector.tensor_copy(dsb[half][:, :], dsb_inv[half][:, :])
                    for g in range(N_CORES):
                        a_raw = t1.tile([128, R], BF16, tag="araw")
                        nc.sync.dma_start(
                            a_raw[0:64, :], a2a_out[2 * half][g, 0:64, :]
                        )
                        nc.sync.dma_start(
                            a_raw[64:128, :], a2a_out[2 * half + 1][g, 0:64, :]
                        )
                        bc = bc_pool.tile(
                            [128, QB], F32, tag=bc_tag, name=f"bc{half}_{g}"
                        )
                        nc.tensor.matmul(
                            bc[:, 0:R],
                            sel_sb[:, 2 * g + half, :],
                            dsb[half][:, :],
                            start=True, stop=True,
                        )
                        nc.vector.tensor_mul(
                            an_sb[:, 2 * g + half, :], a_raw[:], bc[:, 0:R]
                        )
                        yield

                att3 = emit_att(3)
                for unit in att3:
                    if unit == 0:
                        break
                # pair 0 stored: fire chunks 0,1 and prep half-0 an slices
                # (bc matmuls borrow the acc banks, idle with no proj left)
                emit_chunk(0)
                emit_chunk(1)
                emit_den(0)
                emit_den(1)
                an0 = an_half(0, psA, "acc")
                interleave(att3, an0, att_per_proj=2)
                drive(an0)
                emit_chunk(2)
                emit_chunk(3)
                emit_den(2)
                emit_den(3)

                # out projection: nb 0-2 get six accumulators (acc, pv, and
                # both halves of one sc slot) so all their half-0 work runs
                # under the chunk-2/3 transfers; the an-half-1 bc matmuls
                # use the second sc slot; nb3 runs last in the acc slots.
                poA = [psA.tile([128, QB], F32, tag="acc", name=f"poA{q}") for q in range(2)]
                poB = [psPV.tile([128, QB], F32, tag="pv", name=f"poB{q}") for q in range(2)]
                poCt = psB.tile([128, 2, QB], F32, tag="sc", name="poC")
                po_aps = {
                    0: [poA[0][:], poA[1][:]],
                    1: [poB[0][:], poB[1][:]],
                    2: [poCt[:, 0, :], poCt[:, 1, :]],
                }

                def po_mm(nb, q, gh, first, last):
                    nc.tensor.matmul(
                        po_aps[nb][q],
                        an_sb[:, gh, 128 * q : 128 * q + 128],
                        wo_sb[:, gh, 512 * nb : 512 * nb + 512],
                        start=first, stop=last,
                    )

                for g in range(N_CORES):
                    for nb in range(3):
                        for q in range(2):
                            po_mm(nb, q, 2 * g, g == 0, False)
                an1 = an_half(1, psB, "sc")
                for g in range(N_CORES):
                    next(an1, None)
                    for nb in range(3):
                        for q in range(2):
                            po_mm(nb, q, 2 * g + 1, False, g == N_CORES - 1)
                drive(an1)

                def po_evict(nb):
                    for q in range(2):
                        osb = t1.tile([128, QB], F32, tag="osb")
                        nc.vector.tensor_copy(osb[:], po_aps[nb][q])
                        nc.sync.dma_start(
                            out[128 * q : 128 * q + 128, 512 * nb : 512 * nb + 512],
                            osb[:],
                        )

                po_evict(0)
                po_last = [psA.tile([128, QB], F32, tag="acc", name=f"poD{q}") for q in range(2)]
                po_aps[3] = [po_last[0][:], po_last[1][:]]
                for half in range(2):
                    for g in range(N_CORES):
                        gh = 2 * g + half
                        for q in range(2):
                            po_mm(3, q, gh, half == 0 and g == 0,
                                  half == 1 and g == N_CORES - 1)
                po_evict(1)
                po_evict(2)
                po_evict(3)

    nc.compile()
    return nc


_NC_CACHE = None


def _get_nc():
    global _NC_CACHE
    if _NC_CACHE is None:
        _NC_CACHE = _build()
    return _NC_CACHE


def _to_ktile_layout(w):
    m = w.shape[1]
    return np.ascontiguousarray(w.reshape(KT_TILES, 128, m).transpose(1, 0, 2))


def _make_in_maps(x, cos, sin, wq, wk, wv, wo, q_norm_w, k_norm_w):
    x = np.asarray(x, dtype=np.float32)
    cos = np.asarray(cos, dtype=np.float32)
    sin = np.asarray(sin, dtype=np.float32)
    wq = np.asarray(wq, dtype=np.float32)
    wk = np.asarray(wk, dtype=np.float32)
    wv = np.asarray(wv, dtype=np.float32)
    wo = np.asarray(wo, dtype=np.float32)
    qw = np.asarray(q_norm_w, dtype=np.float32)
    kw = np.asarray(k_norm_w, dtype=np.float32)

    xT = np.ascontiguousarray(x[0].T).astype(BF16_NP)
    wo_b = _to_ktile_layout(wo).astype(BF16_NP)

    cosT = cos.T  # [64, SEQ]
    sinT = sin.T
    sgn = np.where(np.arange(64) < 32, -1.0, 1.0).astype(np.float32)
    wrot_q = qw[(np.arange(64) + 32) % 64]
    wrot_k = kw[(np.arange(64) + 32) % 64]
    cq1 = cosT * qw[:, None]
    sq1 = sinT * (sgn * wrot_q)[:, None]
    coswq = np.ascontiguousarray(np.vstack([cq1, cq1])).astype(BF16_NP)
    sinwq = np.ascontiguousarray(np.vstack([sq1, sq1])).astype(BF16_NP)
    coswk = np.ascontiguousarray(cosT * kw[:, None]).astype(BF16_NP)
    sinwk = np.ascontiguousarray(sinT * (sgn * wrot_k)[:, None]).astype(BF16_NP)

    ii, jj = np.meshgrid(np.arange(128), np.arange(128), indexing="ij")
    tri = np.where(ii <= jj, 0.0, NEG).astype(np.float32)
    onesblk = np.zeros((128, 128), np.float32)
    onesblk[0:64, 0:64] = 1.0
    onesblk[64:128, 64:128] = 1.0
    sel16 = np.zeros((2 * N_CORES, 2 * N_CORES, 128), np.float32)
    for g in range(N_CORES):
        for half in range(2):
            for m in range(128):
                sel16[8 * (m // 64) + g, 2 * g + half, m] = 1.0

    in_maps = []
    for c in range(N_CORES):
        wq_c = _to_ktile_layout(
            np.ascontiguousarray(wq[:, 256 * c : 256 * c + 256])
        ).astype(BF16_NP)
        wkv_c = _to_ktile_layout(
            np.ascontiguousarray(
                np.concatenate(
                    [wk[:, 64 * c : 64 * c + 64], wv[:, 64 * c : 64 * c + 64]],
                    axis=1,
                )
            )
        ).astype(BF16_NP)
        in_maps.append(
            {
                "xT": xT,
                "wq": wq_c,
                "wkv": wkv_c,
                "wo": wo_b,
                "coswq": coswq,
                "sinwq": sinwq,
                "coswk": coswk,
                "sinwk": sinwk,
                "tri": tri,
                "sel16": sel16,
                "onesblk": onesblk,
            }
        )
    return in_maps


def kernel(x, cos, sin, wq, wk, wv, wo, q_norm_w, k_norm_w):
    in_maps = _make_in_maps(x, cos, sin, wq, wk, wv, wo, q_norm_w, k_norm_w)
    nc = _get_nc()
    res = run_bass_kernel_spmd(nc, in_maps, core_ids=list(range(N_CORES)))
    rows = [res.results[c]["out"] for c in range(N_CORES)]
    full = np.concatenate(rows, axis=0)  # [SEQ, D_IN]
    return full.reshape(1, SEQ, D_IN).astype(np.float32)
